# revision 9
# baseline (speedup 1.0000x reference)
"""GCNConv (aggregate in in_ch, then @W) + PReLU, distributed over 8 TRN2 NeuronCores.

Decomposition (matches the reference exactly):
    deg[v]  = in-degree of v including self-loop
    dinv    = deg ** -0.5
    xs[u]   = dinv[u] * x[u]                     (per-node src scale, fp16)
    rawagg[v] = sum_{e: dst=v} xs[src_e] + xs[v]
    out     = PReLU(dinv[v] * (rawagg @ W) + b)  (dinv[dst] applied in epilogue)

Sharding: nodes split contiguously over 8 cores (dst ownership); edges routed
host-side to the core owning their destination. Each core:
  1. scales its x shard -> xs shard (fp16), AllGather -> full xs in local DRAM
  2. dma_gather's xs[src] for its ~200k edges (int16 idxs into 25k-row chunk
     views of xs). Gather calls are spread round-robin over the 4 SWDGE
     queues: each queue's descriptors are emitted by a different GpSimd Q7
     core pair (ucode dispatches on cpu_id/2 == queue_num), and queues 1-3
     complete asynchronously at the NX, so 4 emissions run concurrently
     (~4x the single-queue descriptor rate, which is the kernel bottleneck).
  3. S[e, d] = [d == dst_rel_e] one-hot built on DVE (fp16 iota is_equal);
     TensorEngine contracts 128-edge blocks: psum[ch, dst] += G_blk^T @ S_blk
     accumulating one super-window; self-loops enter via xs_win^T @ I.
  4. per-window epilogue: pso = sqrtdeg^T b + aggT^T @ W (PSUM); single ACT
     Prelu op applies out = PReLU(dinv[dst]*pso) with per-partition scale and
     the runtime alpha; DMA out the [12500, 512] shard.
SPMD requires an identical instruction stream on all cores, so per-(sw,chunk)
group sizes are padded to the max over cores, rounded to 128 (pad edges gather
row 0 with dst_rel=-1 so their S row is all zero).
"""

import math

import numpy as np

# Problem constants (hardcoded per the task spec).
N_NODES = 100000
N_EDGES = 1600000
IN_CH = 128
HID_CH = 512
N_CORES = 8

P = 128  # partitions / window size


class Cfg:
    def __init__(self, n_nodes, in_ch, hid_ch, n_cores, chunk, sww):
        assert n_nodes % n_cores == 0
        self.n = n_nodes
        self.in_ch = in_ch
        self.hid = hid_ch
        self.cores = n_cores
        self.nsh = n_nodes // n_cores  # nodes per core
        self.nw = math.ceil(self.nsh / P)  # 128-node windows per core
        self.chunk = chunk  # gather chunk rows (int16 idx range)
        self.nchunk = math.ceil(n_nodes / chunk)
        self.sww = sww  # windows per super-window (PSUM tile width)
        self.nsw = math.ceil(self.nw / sww)
        assert self.nsh % self.nchunk == 0
        self.qsh = self.nsh // self.nchunk  # quarter-shard rows per core


CFG = Cfg(N_NODES, IN_CH, HID_CH, N_CORES, chunk=25000, sww=2)


def route(edge_index, cfg):
    """Host-side edge routing. Returns (dinv, per_core_arrays, structure).

    Edges are grouped per (super-window, chunk); each 128-edge block may mix
    destinations from any window of its super-window (S matrices span the
    whole sww*128-wide PSUM tile). Group sizes are padded to the max over
    cores (SPMD) rounded up to 128.
    """
    src = np.asarray(edge_index[0]).astype(np.int64)
    dst = np.asarray(edge_index[1]).astype(np.int64)

    deg = np.bincount(dst, minlength=cfg.n).astype(np.float64) + 1.0
    dinv = (1.0 / np.sqrt(deg)).astype(np.float32)

    ngrp = cfg.nsw * cfg.nchunk
    core = dst // cfg.nsh
    per_core = []
    counts = np.zeros((cfg.cores, ngrp), dtype=np.int64)
    for c in range(cfg.cores):
        m = core == c
        s = src[m]
        d = dst[m] - c * cfg.nsh
        sw = (d >> 7) // cfg.sww
        # xs_full layout is quarter-interleaved: node v (core sc, offset o,
        # quarter q = o // qsh, within = o % qsh) lives in chunk q at row
        # sc * qsh + within, so gathers for chunk q depend only on the
        # AllGather of every core's quarter q.
        sc = s // cfg.nsh
        o = s % cfg.nsh
        ch = o // cfg.qsh
        g = sw * cfg.nchunk + ch
        counts[c] = np.bincount(g, minlength=ngrp)
        per_core.append((s, d, g, (sc * cfg.qsh + o % cfg.qsh).astype(np.int64)))

    padded = np.ceil(np.max(counts, axis=0) / P).astype(np.int64) * P  # [ngrp]
    # stream order (p8 = group of super-windows, chunk, quad): one gather call
    # covers all quads of a p8 for one chunk
    spc = max(1, 8 // cfg.sww)  # super-windows per gather call (8 windows)
    pairs = (cfg.nsw + spc - 1) // spc
    order = []
    for p8 in range(pairs):
        for ch in range(cfg.nchunk):
            for q in range(spc * p8, min(spc * (p8 + 1), cfg.nsw)):
                order.append(q * cfg.nchunk + ch)
    order = np.array(order, dtype=np.int64)
    sizes = padded[order]
    st2 = np.zeros_like(sizes)
    st2[1:] = np.cumsum(sizes)[:-1]
    starts = np.zeros(ngrp, dtype=np.int64)
    starts[order] = st2
    total = int(padded.sum())

    calls = []
    for p8 in range(pairs):
        for ch in range(cfg.nchunk):
            gs = [
                q * cfg.nchunk + ch
                for q in range(spc * p8, min(spc * (p8 + 1), cfg.nsw))
            ]
            n = int(sum(padded[g] for g in gs))
            if n > 0:
                calls.append(
                    dict(
                        p8=p8,
                        ch=ch,
                        n=n,
                        off=int(starts[gs[0]]),
                        qnb=[int(padded[g]) // P for g in gs],
                    )
                )

    core_arrays = []
    for c in range(cfg.cores):
        s, d, g, row_in_chunk = per_core[c]
        idx = np.zeros(total, dtype=np.int16)
        rel = np.full(total, -1.0, dtype=np.float32)
        ordr = np.argsort(g, kind="stable")
        gs = g[ordr]
        grp_first = np.searchsorted(gs, np.arange(ngrp), "left")
        rank = np.arange(len(gs)) - grp_first[gs]
        slot = starts[gs] + rank
        sw_sorted = gs // cfg.nchunk
        idx[slot] = row_in_chunk[ordr].astype(np.int16)
        rel[slot] = (d[ordr] - sw_sorted * cfg.sww * P).astype(np.float32)
        idx_w = np.tile(idx.reshape(total // 16, 16).T, (8, 1)).copy()
        rel_w = rel.reshape(total // P, P).T.copy()
        core_arrays.append((idx_w, rel_w))

    return dinv, core_arrays, dict(calls=calls, total=total)


def build(cfg, structure):
    import concourse.bass as bass
    import concourse.tile as tile
    from concourse import bacc, mybir

    f32 = mybir.dt.float32
    bf16 = mybir.dt.float16  # fp16: exact small ints, 2x DVE mode
    i16 = mybir.dt.int16
    i32 = mybir.dt.int32
    AF = mybir.ActivationFunctionType
    OP = mybir.AluOpType

    calls = structure["calls"]
    total = structure["total"]

    nsh, nw, hid, in_ch = cfg.nsh, cfg.nw, cfg.hid, cfg.in_ch
    last_rows = nsh - (nw - 1) * P  # rows in the final (partial) window

    nc = bacc.Bacc(
        "TRN2", target_bir_lowering=False, debug=False, num_devices=cfg.cores,
        num_swdge_queues=4,
    )

    x_sh = nc.declare_dram_parameter("x_sh", [nsh, in_ch], f32, isOutput=False)
    dinv_t = nc.declare_dram_parameter("dinv_t", [P, nw], f32, isOutput=False)
    sqdeg_r = nc.declare_dram_parameter("sqdeg_r", [1, nw * P], bf16, isOutput=False)
    w_p = nc.declare_dram_parameter("w_p", [in_ch, hid], f32, isOutput=False)
    b_p = nc.declare_dram_parameter("b_p", [1, hid], f32, isOutput=False)
    alpha_p = nc.declare_dram_parameter("alpha_p", [1, 1], f32, isOutput=False)
    idx_p = nc.declare_dram_parameter("idx_p", [P, total // 16], i16, isOutput=False)
    rel_p = nc.declare_dram_parameter("rel_p", [P, total // P], f32, isOutput=False)
    out_p = nc.declare_dram_parameter("out", [nsh, hid], f32, isOutput=True)

    xs_loc = nc.dram_tensor("xs_loc", [nsh, in_ch], bf16)
    xs_full = nc.dram_tensor("xs_full", [cfg.n, in_ch], bf16, addr_space="Shared")

    with tile.TileContext(nc) as tc:
        with (
            tc.tile_pool(name="const", bufs=1) as constp,
            tc.tile_pool(name="xsbuf", bufs=1) as xsp,
            tc.tile_pool(name="xin", bufs=3) as xinp,
            tc.tile_pool(name="gath", bufs=16) as gp,
            tc.tile_pool(name="smat", bufs=16) as sp,
            tc.tile_pool(name="idxs", bufs=16) as idxp,
            tc.tile_pool(name="aggt", bufs=6) as aggp,
            tc.tile_pool(name="epi", bufs=4) as epip,
            tc.tile_pool(name="sqw", bufs=3) as sqwp,
            tc.tile_pool(name="psw", bufs=5, space="PSUM") as pswp,
            tc.tile_pool(name="pso", bufs=2, space="PSUM") as psop,
            tc.tile_pool(name="psa", bufs=1, space="PSUM") as psap,
        ):
            # ---- constants / setup ----
            wbc = cfg.sww * P
            iota_i = constp.tile([P, wbc], i32)
            nc.gpsimd.iota(iota_i[:], pattern=[[1, wbc]], base=0, channel_multiplier=0)
            iota_f = constp.tile([P, wbc], mybir.dt.float16)
            nc.vector.tensor_copy(iota_f[:], iota_i[:])
            lane_i = constp.tile([P, 1], i32)
            nc.gpsimd.iota(lane_i[:], pattern=[[1, 1]], base=0, channel_multiplier=1)
            lane_f = constp.tile([P, 1], f32)
            nc.vector.tensor_copy(lane_f[:], lane_i[:])
            ident_bf = constp.tile([P, P], bf16)
            nc.vector.tensor_scalar(
                ident_bf[:], iota_f[:, :P], lane_f[:], None, OP.is_equal
            )

            w_f32 = constp.tile([in_ch, hid], f32)
            nc.sync.dma_start(w_f32[:], w_p[:])
            w_bf = constp.tile([in_ch, hid], bf16)
            nc.vector.tensor_copy(w_bf[:], w_f32[:])

            b_f32 = constp.tile([1, hid], f32)
            nc.sync.dma_start(b_f32[:], b_p[:])
            b_bf = constp.tile([1, hid], bf16)
            nc.vector.tensor_copy(b_bf[:], b_f32[:])

            ones1 = constp.tile([1, P], f32)
            nc.vector.memset(ones1[:], 1.0)
            alpha_sb = constp.tile([1, 1], f32)
            nc.sync.dma_start(alpha_sb[:], alpha_p[:])

            dinv_sb = constp.tile([P, nw], f32)
            nc.sync.dma_start(dinv_sb[:], dinv_t[:])

            # alpha broadcast to [128,1] via K=1 matmul with ones
            psum_a = psap.tile([P, 1], f32, space="PSUM")
            nc.tensor.matmul(
                psum_a[:], lhsT=ones1[:], rhs=alpha_sb[:], start=True, stop=True
            )
            alpha_bc = constp.tile([P, 1], f32)
            nc.vector.tensor_copy(alpha_bc[:], psum_a[:])

            # ---- phase 1: xs = dinv * x ; AllGather ----
            xs_sb = xsp.tile([P, nw, in_ch], bf16)
            WG = 4
            nfull = (nw - 1) if last_rows < P else nw  # full 128-row windows
            wg = 0
            while wg < nfull - (nfull % WG):
                g = WG
                xt = xinp.tile([P, g, in_ch], f32, tag="xt")
                nc.sync.dma_start(
                    xt[:],
                    x_sh[wg * P : (wg + g) * P, :].rearrange(
                        "(g p) c -> p g c", p=P
                    ),
                )
                for j in range(g):
                    nc.vector.tensor_scalar(
                        xs_sb[:, wg + j, :],
                        xt[:, j, :],
                        dinv_sb[:, wg + j : wg + j + 1],
                        None,
                        OP.mult,
                    )
                nc.sync.dma_start(
                    xs_loc[wg * P : (wg + g) * P, :].rearrange(
                        "(g p) c -> p g c", p=P
                    ),
                    xs_sb[:, wg : wg + g, :],
                )
                wg += g
            for w in range(wg, nw):
                rows = P if w < nw - 1 else last_rows
                xt = xinp.tile([P, 1, in_ch], f32, tag="xt")
                if rows < P:
                    nc.vector.memset(xt[:], 0.0)
                nc.sync.dma_start(xt[:rows, 0, :], x_sh[w * P : w * P + rows, :])
                nc.vector.tensor_scalar(
                    xs_sb[:, w, :], xt[:, 0, :], dinv_sb[:, w : w + 1], None, OP.mult
                )
                nc.sync.dma_start(
                    xs_loc[w * P : w * P + rows, :], xs_sb[:rows, w, :]
                )

            # quartered AllGather: chunk q of xs_full = concat over cores of
            # each core's quarter q; gathers for chunk q wait only on AG_q
            for q in range(cfg.nchunk):
                nc.gpsimd.collective_compute(
                    "AllGather",
                    mybir.AluOpType.bypass,
                    replica_groups=[list(range(cfg.cores))],
                    ins=[xs_loc[q * cfg.qsh : (q + 1) * cfg.qsh, :]],
                    outs=[xs_full[q * cfg.chunk : (q + 1) * cfg.chunk, :]],
                )

            # ---- phase 2: gather + aggregate + epilogue ----
            wb = cfg.sww * P  # S width (<= one PSUM tile of dst cols)
            spc = max(1, 8 // cfg.sww)
            pairs = (cfg.nsw + spc - 1) // spc
            calls_by_p8 = {}
            for cl in calls:
                calls_by_p8.setdefault(cl["p8"], []).append(cl)

            def emit_epilogue(psw, ws):
                for wi, w in enumerate(ws):
                    rows = P if w < nw - 1 else last_rows
                    aggt = aggp.tile([P, P], bf16, tag="aggt")
                    nc.vector.tensor_copy(aggt[:], psw[:, wi * P : (wi + 1) * P])
                    sqw = sqwp.tile([1, P], bf16, tag="sq")
                    nc.sync.dma_start(sqw[:], sqdeg_r[:, w * P : (w + 1) * P])
                    pso = psop.tile([P, hid], f32, space="PSUM", tag="pso")
                    nc.tensor.matmul(
                        pso[:], lhsT=sqw[:], rhs=b_bf[:],
                        start=True, stop=False,
                    )
                    nc.tensor.matmul(
                        pso[:], lhsT=aggt[:], rhs=w_bf[:], start=False, stop=True
                    )
                    ot = epip.tile([P, hid], f32, tag="ot")
                    nc.scalar.activation(
                        ot[:], pso[:], AF.Prelu,
                        bias=0.0, scale=dinv_sb[:, w : w + 1], alpha=alpha_bc[:, :1],
                    )
                    nc.sync.dma_start(out_p[w * P : w * P + rows, :], ot[:rows, :])

            for p8 in range(pairs):
                p8_calls = calls_by_p8.get(p8, [])
                # Split each (p8, chunk) call into two half-calls (quads 0-1 /
                # quads 2-3) and dispatch round-robin over the 4 SWDGE queues
                # (queue = chunk): each queue's descriptors are emitted by a
                # different Q7 pair, queue 0 blocks the Pool NX and paces
                # dispatch while 1-3 emit asynchronously; halving the calls
                # halves the convoy wait when a pair is still busy.
                halves = []  # (cl, half, off, n, qnb_half)
                for cl in p8_calls:
                    qnb = cl["qnb"]
                    for h in (0, 1):
                        qh = qnb[2 * h : 2 * h + 2]
                        nh = sum(qh) * P
                        if nh == 0:
                            continue
                        offh = cl["off"] + sum(qnb[: 2 * h]) * P
                        halves.append((cl, h, offh, nh, qh))
                halves.sort(key=lambda t: (t[1], t[0]["ch"] == 0, t[0]["ch"]))
                seg_by_ch = {}
                for cl, h, offh, nh, qh in halves:
                    nb = nh // P
                    it = idxp.tile([P, nh // 16], i16, tag="idx")
                    nc.sync.dma_start(
                        it[:], idx_p[:, offh // 16 : (offh + nh) // 16]
                    )
                    gt = gp.tile([P, nb, in_ch], bf16, tag="g")
                    ch0 = cl["ch"] * cfg.chunk
                    ch1 = min(ch0 + cfg.chunk, cfg.n)
                    nc.gpsimd.dma_gather(
                        gt[:], xs_full[ch0:ch1, :], it[:], nh, nh, in_ch,
                        single_packet=False, queue_num=cl["ch"],
                    )
                    rel_sb = idxp.tile([P, nb], f32, tag="rel")
                    nc.sync.dma_start(
                        rel_sb[:], rel_p[:, offh // P : offh // P + nb]
                    )
                    seg_by_ch.setdefault(cl["ch"], {})[h] = (gt, rel_sb, qh)

                nquad = min(spc, cfg.nsw - spc * p8)
                disp_order = [c for c in (1, 2, 3, 0) if c in seg_by_ch]
                # per-quad psum tiles + self-loop injection; only the FIRST
                # matmul of a tile sets start=True (zero_accum zeroes it)
                qinfo = []
                for qi in range(nquad):
                    sw = spc * p8 + qi
                    ws = list(range(sw * cfg.sww, min((sw + 1) * cfg.sww, nw)))
                    h, hq = divmod(qi, 2)
                    nmm = sum(
                        s[h][2][hq]
                        for s in seg_by_ch.values()
                        if h in s and hq < len(s[h][2])
                    )
                    psw = pswp.tile([P, wb], f32, space="PSUM", tag="psw")
                    for wi, w in enumerate(ws):
                        nc.tensor.matmul(
                            psw[:, wi * P : (wi + 1) * P],
                            lhsT=xs_sb[:, w, :],
                            rhs=ident_bf[:],
                            start=(wi == 0),
                            stop=(nmm == 0 and wi == len(ws) - 1),
                            skip_group_check=True,
                        )
                    if nmm == 0:
                        emit_epilogue(psw, ws)
                    qinfo.append(dict(ws=ws, nmm=nmm, psw=psw, k=0))
                # consume S-blocks in gather-landing order (half, then chunk
                # dispatch order) to avoid DVE head-of-line waits; epilogue
                # fires as soon as a quad's accumulation completes
                for h in (0, 1):
                    for c in disp_order:
                        if h not in seg_by_ch[c]:
                            continue
                        gt, rel_sb, qh = seg_by_ch[c][h]
                        for hq in range(len(qh)):
                            qi = 2 * h + hq
                            if qi >= nquad:
                                continue
                            info = qinfo[qi]
                            psw, ws = info["psw"], info["ws"]
                            wsw = len(ws) * P
                            qoff = sum(qh[:hq])
                            for bi in range(qoff, qoff + qh[hq]):
                                st = sp.tile([P, wsw], bf16, tag="s")
                                nc.vector.tensor_scalar(
                                    st[:],
                                    iota_f[:, :wsw],
                                    rel_sb[:, bi : bi + 1],
                                    None,
                                    OP.is_equal,
                                )
                                info["k"] += 1
                                nc.tensor.matmul(
                                    psw[:, :wsw],
                                    lhsT=gt[:, bi, :],
                                    rhs=st[:],
                                    start=False,
                                    stop=(info["k"] == info["nmm"]),
                                    skip_group_check=True,
                                )
                            if info["k"] == info["nmm"] and info["nmm"] > 0:
                                emit_epilogue(psw, ws)

    nc.compile()
    return nc


def _prep_inputs(x, edge_index, W, b, alpha, cfg):
    dinv, core_arrays, structure = route(edge_index, cfg)
    x = np.asarray(x, dtype=np.float32)
    W = np.asarray(W, dtype=np.float32)
    b = np.asarray(b, dtype=np.float32).reshape(1, cfg.hid)
    alpha = np.asarray(alpha, dtype=np.float32).reshape(1, 1)

    pad_n = cfg.nw * P - cfg.nsh
    in_maps = []
    for c in range(cfg.cores):
        idx_w, rel_w = core_arrays[c]
        dsh = dinv[c * cfg.nsh : (c + 1) * cfg.nsh]
        dsh = np.concatenate([dsh, np.ones(pad_n, np.float32)])
        in_maps.append(
            {
                "x_sh": x[c * cfg.nsh : (c + 1) * cfg.nsh],
                "dinv_t": dsh.reshape(cfg.nw, P).T.copy(),
                "sqdeg_r": (1.0 / dsh).reshape(1, cfg.nw * P).astype(np.float16),
                "w_p": W,
                "b_p": b,
                "alpha_p": alpha,
                "idx_p": idx_w,
                "rel_p": rel_w,
            }
        )
    return in_maps, structure


def kernel(x, edge_index, W, b, alpha):
    from concourse.bass_utils import run_bass_kernel_spmd

    cfg = CFG
    in_maps, structure = _prep_inputs(x, edge_index, W, b, alpha, cfg)
    nc = build(cfg, structure)
    res = run_bass_kernel_spmd(nc, in_maps, list(range(cfg.cores)))
    out = np.concatenate(
        [np.asarray(res.results[c]["out"]) for c in range(cfg.cores)], axis=0
    )
    return out.astype(np.float32)


# revision 11
# speedup vs baseline: 1.0840x; 1.0840x over previous
"""GCNConv (aggregate in in_ch, then @W) + PReLU, distributed over 8 TRN2 NeuronCores.

Decomposition (matches the reference exactly):
    deg[v]  = in-degree of v including self-loop
    dinv    = deg ** -0.5
    xs[u]   = dinv[u] * x[u]                     (per-node src scale, fp16)
    rawagg[v] = sum_{e: dst=v} xs[src_e] + xs[v]
    out     = PReLU(dinv[v] * (rawagg @ W) + b)  (dinv[dst] applied in epilogue)

Sharding: nodes split contiguously over 8 cores (dst ownership); edges routed
host-side to the core owning their destination. Each core:
  1. scales its x shard -> xs shard (fp16), AllGather -> full xs in local DRAM
  2. dma_gather's xs[src] for its ~200k edges (int16 idxs into 25k-row chunk
     views of xs). Gather calls are spread round-robin over the 4 SWDGE
     queues: each queue's descriptors are emitted by a different GpSimd Q7
     core pair (ucode dispatches on cpu_id/2 == queue_num), and queues 1-3
     complete asynchronously at the NX, so 4 emissions run concurrently
     (~4x the single-queue descriptor rate, which is the kernel bottleneck).
  3. S[e, d] = [d == dst_rel_e] one-hot built on DVE (fp16 iota is_equal);
     TensorEngine contracts 128-edge blocks: psum[ch, dst] += G_blk^T @ S_blk
     accumulating one super-window; self-loops enter via xs_win^T @ I.
  4. per-window epilogue: pso = sqrtdeg^T b + aggT^T @ W (PSUM); single ACT
     Prelu op applies out = PReLU(dinv[dst]*pso) with per-partition scale and
     the runtime alpha; DMA out the [12500, 512] shard.
SPMD requires an identical instruction stream on all cores, so per-(sw,chunk)
group sizes are padded to the max over cores, rounded to 128 (pad edges gather
row 0 with dst_rel=-1 so their S row is all zero).
"""

import math

import numpy as np

# Problem constants (hardcoded per the task spec).
N_NODES = 100000
N_EDGES = 1600000
IN_CH = 128
HID_CH = 512
N_CORES = 8

P = 128  # partitions / window size


class Cfg:
    def __init__(self, n_nodes, in_ch, hid_ch, n_cores, chunk, sww):
        assert n_nodes % n_cores == 0
        self.n = n_nodes
        self.in_ch = in_ch
        self.hid = hid_ch
        self.cores = n_cores
        self.nsh = n_nodes // n_cores  # nodes per core
        self.nw = math.ceil(self.nsh / P)  # 128-node windows per core
        self.chunk = chunk  # gather chunk rows (int16 idx range)
        self.nchunk = math.ceil(n_nodes / chunk)
        self.sww = sww  # windows per super-window (PSUM tile width)
        self.nsw = math.ceil(self.nw / sww)
        assert self.nsh % self.nchunk == 0
        self.qsh = self.nsh // self.nchunk  # quarter-shard rows per core


CFG = Cfg(N_NODES, IN_CH, HID_CH, N_CORES, chunk=25000, sww=2)


def route(edge_index, cfg):
    """Host-side edge routing. Returns (dinv, per_core_arrays, structure).

    Edges are grouped per (super-window, chunk); each 128-edge block may mix
    destinations from any window of its super-window (S matrices span the
    whole sww*128-wide PSUM tile). Group sizes are padded to the max over
    cores (SPMD) rounded up to 128.
    """
    src = np.asarray(edge_index[0]).astype(np.int64)
    dst = np.asarray(edge_index[1]).astype(np.int64)

    deg = np.bincount(dst, minlength=cfg.n).astype(np.float64) + 1.0
    dinv = (1.0 / np.sqrt(deg)).astype(np.float32)

    ngrp = cfg.nsw * cfg.nchunk
    core = dst // cfg.nsh
    per_core = []
    counts = np.zeros((cfg.cores, ngrp), dtype=np.int64)
    for c in range(cfg.cores):
        m = core == c
        s = src[m]
        d = dst[m] - c * cfg.nsh
        sw = (d >> 7) // cfg.sww
        # xs_full layout is quarter-interleaved: node v (core sc, offset o,
        # quarter q = o // qsh, within = o % qsh) lives in chunk q at row
        # sc * qsh + within, so gathers for chunk q depend only on the
        # AllGather of every core's quarter q.
        sc = s // cfg.nsh
        o = s % cfg.nsh
        ch = o // cfg.qsh
        g = sw * cfg.nchunk + ch
        counts[c] = np.bincount(g, minlength=ngrp)
        per_core.append((s, d, g, (sc * cfg.qsh + o % cfg.qsh).astype(np.int64)))

    padded = np.ceil(np.max(counts, axis=0) / P).astype(np.int64) * P  # [ngrp]
    # stream order (p8 = group of super-windows, chunk, quad): one gather call
    # covers all quads of a p8 for one chunk
    spc = max(1, 8 // cfg.sww)  # super-windows per gather call (8 windows)
    pairs = (cfg.nsw + spc - 1) // spc
    order = []
    for p8 in range(pairs):
        for ch in range(cfg.nchunk):
            for q in range(spc * p8, min(spc * (p8 + 1), cfg.nsw)):
                order.append(q * cfg.nchunk + ch)
    order = np.array(order, dtype=np.int64)
    sizes = padded[order]
    st2 = np.zeros_like(sizes)
    st2[1:] = np.cumsum(sizes)[:-1]
    starts = np.zeros(ngrp, dtype=np.int64)
    starts[order] = st2
    total = int(padded.sum())

    calls = []
    for p8 in range(pairs):
        for ch in range(cfg.nchunk):
            gs = [
                q * cfg.nchunk + ch
                for q in range(spc * p8, min(spc * (p8 + 1), cfg.nsw))
            ]
            n = int(sum(padded[g] for g in gs))
            if n > 0:
                calls.append(
                    dict(
                        p8=p8,
                        ch=ch,
                        n=n,
                        off=int(starts[gs[0]]),
                        qnb=[int(padded[g]) // P for g in gs],
                    )
                )

    core_arrays = []
    for c in range(cfg.cores):
        s, d, g, row_in_chunk = per_core[c]
        idx = np.zeros(total, dtype=np.int16)
        rel = np.full(total, -1.0, dtype=np.float32)
        ordr = np.argsort(g, kind="stable")
        gs = g[ordr]
        grp_first = np.searchsorted(gs, np.arange(ngrp), "left")
        rank = np.arange(len(gs)) - grp_first[gs]
        slot = starts[gs] + rank
        sw_sorted = gs // cfg.nchunk
        idx[slot] = row_in_chunk[ordr].astype(np.int16)
        rel[slot] = (d[ordr] - sw_sorted * cfg.sww * P).astype(np.float32)
        idx_w = np.tile(idx.reshape(total // 16, 16).T, (8, 1)).copy()
        rel_w = rel.reshape(total // P, P).T.copy()
        core_arrays.append((idx_w, rel_w))

    return dinv, core_arrays, dict(calls=calls, total=total)


def build(cfg, structure):
    import concourse.bass as bass
    import concourse.tile as tile
    from concourse import bacc, mybir

    f32 = mybir.dt.float32
    bf16 = mybir.dt.float16  # fp16: exact small ints, 2x DVE mode
    i16 = mybir.dt.int16
    i32 = mybir.dt.int32
    AF = mybir.ActivationFunctionType
    OP = mybir.AluOpType

    calls = structure["calls"]
    total = structure["total"]

    nsh, nw, hid, in_ch = cfg.nsh, cfg.nw, cfg.hid, cfg.in_ch
    last_rows = nsh - (nw - 1) * P  # rows in the final (partial) window

    nc = bacc.Bacc(
        "TRN2", target_bir_lowering=False, debug=False, num_devices=cfg.cores,
        num_swdge_queues=4,
    )

    x_sh = nc.declare_dram_parameter("x_sh", [nsh, in_ch], f32, isOutput=False)
    dinv_t = nc.declare_dram_parameter("dinv_t", [P, nw], f32, isOutput=False)
    sqdeg_r = nc.declare_dram_parameter("sqdeg_r", [1, nw * P], bf16, isOutput=False)
    w_p = nc.declare_dram_parameter("w_p", [in_ch, hid], f32, isOutput=False)
    b_p = nc.declare_dram_parameter("b_p", [1, hid], f32, isOutput=False)
    alpha_p = nc.declare_dram_parameter("alpha_p", [1, 1], f32, isOutput=False)
    idx_p = nc.declare_dram_parameter("idx_p", [P, total // 16], i16, isOutput=False)
    rel_p = nc.declare_dram_parameter("rel_p", [P, total // P], f32, isOutput=False)
    out_p = nc.declare_dram_parameter("out", [nsh, hid], f32, isOutput=True)

    xs_loc = nc.dram_tensor("xs_loc", [nsh, in_ch], bf16)
    xs_full = nc.dram_tensor("xs_full", [cfg.n, in_ch], bf16, addr_space="Shared")

    with tile.TileContext(nc) as tc:
        with (
            tc.tile_pool(name="const", bufs=1) as constp,
            tc.tile_pool(name="xsbuf", bufs=1) as xsp,
            tc.tile_pool(name="xin", bufs=3) as xinp,
            tc.tile_pool(name="gath", bufs=16) as gp,
            tc.tile_pool(name="smat", bufs=16) as sp,
            tc.tile_pool(name="idxs", bufs=16) as idxp,
            tc.tile_pool(name="aggt", bufs=6) as aggp,
            tc.tile_pool(name="epi", bufs=4) as epip,
            tc.tile_pool(name="sqw", bufs=3) as sqwp,
            tc.tile_pool(name="psw", bufs=5, space="PSUM") as pswp,
            tc.tile_pool(name="pso", bufs=2, space="PSUM") as psop,
            tc.tile_pool(name="psa", bufs=1, space="PSUM") as psap,
        ):
            # ---- constants / setup ----
            wbc = cfg.sww * P
            iota_i = constp.tile([P, wbc], i32)
            nc.gpsimd.iota(iota_i[:], pattern=[[1, wbc]], base=0, channel_multiplier=0)
            iota_f = constp.tile([P, wbc], mybir.dt.float16)
            nc.vector.tensor_copy(iota_f[:], iota_i[:])
            lane_i = constp.tile([P, 1], i32)
            nc.gpsimd.iota(lane_i[:], pattern=[[1, 1]], base=0, channel_multiplier=1)
            lane_f = constp.tile([P, 1], f32)
            nc.vector.tensor_copy(lane_f[:], lane_i[:])
            ident_bf = constp.tile([P, P], bf16)
            nc.vector.tensor_scalar(
                ident_bf[:], iota_f[:, :P], lane_f[:], None, OP.is_equal
            )

            w_f32 = constp.tile([in_ch, hid], f32)
            nc.sync.dma_start(w_f32[:], w_p[:])
            w_bf = constp.tile([in_ch, hid], bf16)
            nc.vector.tensor_copy(w_bf[:], w_f32[:])

            b_f32 = constp.tile([1, hid], f32)
            nc.sync.dma_start(b_f32[:], b_p[:])
            b_bf = constp.tile([1, hid], bf16)
            nc.vector.tensor_copy(b_bf[:], b_f32[:])

            ones1 = constp.tile([1, P], f32)
            nc.vector.memset(ones1[:], 1.0)
            alpha_sb = constp.tile([1, 1], f32)
            nc.sync.dma_start(alpha_sb[:], alpha_p[:])

            dinv_sb = constp.tile([P, nw], f32)
            nc.sync.dma_start(dinv_sb[:], dinv_t[:])

            # alpha broadcast to [128,1] via K=1 matmul with ones
            psum_a = psap.tile([P, 1], f32, space="PSUM")
            nc.tensor.matmul(
                psum_a[:], lhsT=ones1[:], rhs=alpha_sb[:], start=True, stop=True
            )
            alpha_bc = constp.tile([P, 1], f32)
            nc.vector.tensor_copy(alpha_bc[:], psum_a[:])

            # ---- phase 1: xs = dinv * x ; AllGather ----
            xs_sb = xsp.tile([P, nw, in_ch], bf16)
            WG = 4
            nfull = (nw - 1) if last_rows < P else nw  # full 128-row windows
            wg = 0
            while wg < nfull - (nfull % WG):
                g = WG
                xt = xinp.tile([P, g, in_ch], f32, tag="xt")
                nc.scalar.dma_start(
                    xt[:],
                    x_sh[wg * P : (wg + g) * P, :].rearrange(
                        "(g p) c -> p g c", p=P
                    ),
                )
                for j in range(g):
                    nc.vector.tensor_scalar(
                        xs_sb[:, wg + j, :],
                        xt[:, j, :],
                        dinv_sb[:, wg + j : wg + j + 1],
                        None,
                        OP.mult,
                    )
                nc.scalar.dma_start(
                    xs_loc[wg * P : (wg + g) * P, :].rearrange(
                        "(g p) c -> p g c", p=P
                    ),
                    xs_sb[:, wg : wg + g, :],
                )
                wg += g
            for w in range(wg, nw):
                rows = P if w < nw - 1 else last_rows
                xt = xinp.tile([P, 1, in_ch], f32, tag="xt")
                if rows < P:
                    nc.vector.memset(xt[:], 0.0)
                nc.scalar.dma_start(xt[:rows, 0, :], x_sh[w * P : w * P + rows, :])
                nc.vector.tensor_scalar(
                    xs_sb[:, w, :], xt[:, 0, :], dinv_sb[:, w : w + 1], None, OP.mult
                )
                nc.scalar.dma_start(
                    xs_loc[w * P : w * P + rows, :], xs_sb[:rows, w, :]
                )

            # quartered AllGather: chunk q of xs_full = concat over cores of
            # each core's quarter q; gathers for chunk q wait only on AG_q
            for q in range(cfg.nchunk):
                nc.gpsimd.collective_compute(
                    "AllGather",
                    mybir.AluOpType.bypass,
                    replica_groups=[list(range(cfg.cores))],
                    ins=[xs_loc[q * cfg.qsh : (q + 1) * cfg.qsh, :]],
                    outs=[xs_full[q * cfg.chunk : (q + 1) * cfg.chunk, :]],
                )

            # ---- phase 2: gather + aggregate + epilogue ----
            wb = cfg.sww * P  # S width (<= one PSUM tile of dst cols)
            spc = max(1, 8 // cfg.sww)
            pairs = (cfg.nsw + spc - 1) // spc
            calls_by_p8 = {}
            for cl in calls:
                calls_by_p8.setdefault(cl["p8"], []).append(cl)

            def emit_epilogue(psw, ws):
                for wi, w in enumerate(ws):
                    rows = P if w < nw - 1 else last_rows
                    aggt = aggp.tile([P, P], bf16, tag="aggt")
                    nc.vector.tensor_copy(aggt[:], psw[:, wi * P : (wi + 1) * P])
                    sqw = sqwp.tile([1, P], bf16, tag="sq")
                    nc.sync.dma_start(sqw[:], sqdeg_r[:, w * P : (w + 1) * P])
                    pso = psop.tile([P, hid], f32, space="PSUM", tag="pso")
                    nc.tensor.matmul(
                        pso[:], lhsT=sqw[:], rhs=b_bf[:],
                        start=True, stop=False,
                    )
                    nc.tensor.matmul(
                        pso[:], lhsT=aggt[:], rhs=w_bf[:], start=False, stop=True
                    )
                    ot = epip.tile([P, hid], f32, tag="ot")
                    nc.scalar.activation(
                        ot[:], pso[:], AF.Prelu,
                        bias=0.0, scale=dinv_sb[:, w : w + 1], alpha=alpha_bc[:, :1],
                    )
                    nc.scalar.dma_start(out_p[w * P : w * P + rows, :], ot[:rows, :])

            for p8 in range(pairs):
                p8_calls = calls_by_p8.get(p8, [])
                # Split each (p8, chunk) call into two half-calls (quads 0-1 /
                # quads 2-3) and dispatch round-robin over the 4 SWDGE queues
                # (queue = chunk): each queue's descriptors are emitted by a
                # different Q7 pair, queue 0 blocks the Pool NX and paces
                # dispatch while 1-3 emit asynchronously; halving the calls
                # halves the convoy wait when a pair is still busy.
                halves = []  # (cl, half, off, n, qnb_half)
                for cl in p8_calls:
                    qnb = cl["qnb"]
                    for h in (0, 1):
                        qh = qnb[2 * h : 2 * h + 2]
                        nh = sum(qh) * P
                        if nh == 0:
                            continue
                        offh = cl["off"] + sum(qnb[: 2 * h]) * P
                        halves.append((cl, h, offh, nh, qh))
                halves.sort(key=lambda t: (t[1], t[0]["ch"] == 0, t[0]["ch"]))
                seg_by_ch = {}
                for cl, h, offh, nh, qh in halves:
                    nb = nh // P
                    it = idxp.tile([P, nh // 16], i16, tag="idx")
                    nc.sync.dma_start(
                        it[:], idx_p[:, offh // 16 : (offh + nh) // 16]
                    )
                    gt = gp.tile([P, nb, in_ch], bf16, tag="g")
                    ch0 = cl["ch"] * cfg.chunk
                    ch1 = min(ch0 + cfg.chunk, cfg.n)
                    nc.gpsimd.dma_gather(
                        gt[:], xs_full[ch0:ch1, :], it[:], nh, nh, in_ch,
                        single_packet=False, queue_num=cl["ch"],
                    )
                    rel_sb = idxp.tile([P, nb], f32, tag="rel")
                    nc.sync.dma_start(
                        rel_sb[:], rel_p[:, offh // P : offh // P + nb]
                    )
                    seg_by_ch.setdefault(cl["ch"], {})[h] = (gt, rel_sb, qh)

                nquad = min(spc, cfg.nsw - spc * p8)
                disp_order = [c for c in (1, 2, 3, 0) if c in seg_by_ch]
                # per-quad psum tiles + self-loop injection; only the FIRST
                # matmul of a tile sets start=True (zero_accum zeroes it)
                qinfo = []
                for qi in range(nquad):
                    sw = spc * p8 + qi
                    ws = list(range(sw * cfg.sww, min((sw + 1) * cfg.sww, nw)))
                    h, hq = divmod(qi, 2)
                    nmm = sum(
                        s[h][2][hq]
                        for s in seg_by_ch.values()
                        if h in s and hq < len(s[h][2])
                    )
                    psw = pswp.tile([P, wb], f32, space="PSUM", tag="psw")
                    for wi, w in enumerate(ws):
                        nc.tensor.matmul(
                            psw[:, wi * P : (wi + 1) * P],
                            lhsT=xs_sb[:, w, :],
                            rhs=ident_bf[:],
                            start=(wi == 0),
                            stop=(nmm == 0 and wi == len(ws) - 1),
                            skip_group_check=True,
                        )
                    if nmm == 0:
                        emit_epilogue(psw, ws)
                    qinfo.append(dict(ws=ws, nmm=nmm, psw=psw, k=0))
                # consume S-blocks in gather-landing order (half, then chunk
                # dispatch order) to avoid DVE head-of-line waits; epilogue
                # fires as soon as a quad's accumulation completes
                for h in (0, 1):
                    for c in disp_order:
                        if h not in seg_by_ch[c]:
                            continue
                        gt, rel_sb, qh = seg_by_ch[c][h]
                        for hq in range(len(qh)):
                            qi = 2 * h + hq
                            if qi >= nquad:
                                continue
                            info = qinfo[qi]
                            psw, ws = info["psw"], info["ws"]
                            wsw = len(ws) * P
                            qoff = sum(qh[:hq])
                            for bi in range(qoff, qoff + qh[hq]):
                                st = sp.tile([P, wsw], bf16, tag="s")
                                nc.vector.tensor_scalar(
                                    st[:],
                                    iota_f[:, :wsw],
                                    rel_sb[:, bi : bi + 1],
                                    None,
                                    OP.is_equal,
                                )
                                info["k"] += 1
                                nc.tensor.matmul(
                                    psw[:, :wsw],
                                    lhsT=gt[:, bi, :],
                                    rhs=st[:],
                                    start=False,
                                    stop=(info["k"] == info["nmm"]),
                                    skip_group_check=True,
                                )
                            if info["k"] == info["nmm"] and info["nmm"] > 0:
                                emit_epilogue(psw, ws)

    nc.compile()
    return nc


def _prep_inputs(x, edge_index, W, b, alpha, cfg):
    dinv, core_arrays, structure = route(edge_index, cfg)
    x = np.asarray(x, dtype=np.float32)
    W = np.asarray(W, dtype=np.float32)
    b = np.asarray(b, dtype=np.float32).reshape(1, cfg.hid)
    alpha = np.asarray(alpha, dtype=np.float32).reshape(1, 1)

    pad_n = cfg.nw * P - cfg.nsh
    in_maps = []
    for c in range(cfg.cores):
        idx_w, rel_w = core_arrays[c]
        dsh = dinv[c * cfg.nsh : (c + 1) * cfg.nsh]
        dsh = np.concatenate([dsh, np.ones(pad_n, np.float32)])
        in_maps.append(
            {
                "x_sh": x[c * cfg.nsh : (c + 1) * cfg.nsh],
                "dinv_t": dsh.reshape(cfg.nw, P).T.copy(),
                "sqdeg_r": (1.0 / dsh).reshape(1, cfg.nw * P).astype(np.float16),
                "w_p": W,
                "b_p": b,
                "alpha_p": alpha,
                "idx_p": idx_w,
                "rel_p": rel_w,
            }
        )
    return in_maps, structure


def kernel(x, edge_index, W, b, alpha):
    from concourse.bass_utils import run_bass_kernel_spmd

    cfg = CFG
    in_maps, structure = _prep_inputs(x, edge_index, W, b, alpha, cfg)
    nc = build(cfg, structure)
    res = run_bass_kernel_spmd(nc, in_maps, list(range(cfg.cores)))
    out = np.concatenate(
        [np.asarray(res.results[c]["out"]) for c in range(cfg.cores)], axis=0
    )
    return out.astype(np.float32)


# revision 12
# speedup vs baseline: 1.3265x; 1.2237x over previous
"""GCNConv (aggregate in in_ch, then @W) + PReLU, distributed over 8 TRN2 NeuronCores.

Decomposition (matches the reference exactly):
    deg[v]  = in-degree of v including self-loop
    dinv    = deg ** -0.5
    xs[u]   = dinv[u] * x[u]                     (per-node src scale, fp16)
    rawagg[v] = sum_{e: dst=v} xs[src_e] + xs[v]
    out     = PReLU(dinv[v] * (rawagg @ W) + b)  (dinv[dst] applied in epilogue)

Sharding: nodes split contiguously over 8 cores (dst ownership); edges routed
host-side to the core owning their destination. Each core:
  1. scales its x shard -> xs shard (fp16), AllGather -> full xs in local DRAM
  2. dma_gather's xs[src] for its ~200k edges (int16 idxs into 25k-row chunk
     views of xs). Gather calls are spread round-robin over the 4 SWDGE
     queues: each queue's descriptors are emitted by a different GpSimd Q7
     core pair (ucode dispatches on cpu_id/2 == queue_num), and queues 1-3
     complete asynchronously at the NX, so 4 emissions run concurrently
     (~4x the single-queue descriptor rate, which is the kernel bottleneck).
  3. S[e, d] = [d == dst_rel_e] one-hot built on DVE (fp16 iota is_equal);
     TensorEngine contracts 128-edge blocks: psum[ch, dst] += G_blk^T @ S_blk
     accumulating one super-window; self-loops enter via xs_win^T @ I.
  4. per-window epilogue: pso = sqrtdeg^T b + aggT^T @ W (PSUM); single ACT
     Prelu op applies out = PReLU(dinv[dst]*pso) with per-partition scale and
     the runtime alpha; DMA out the [12500, 512] shard.
SPMD requires an identical instruction stream on all cores, so per-(sw,chunk)
group sizes are padded to the max over cores, rounded to 128 (pad edges gather
row 0 with dst_rel=-1 so their S row is all zero).
"""

import math

import numpy as np

# Problem constants (hardcoded per the task spec).
N_NODES = 100000
N_EDGES = 1600000
IN_CH = 128
HID_CH = 512
N_CORES = 8

P = 128  # partitions / window size


class Cfg:
    def __init__(self, n_nodes, in_ch, hid_ch, n_cores, chunk, sww):
        assert n_nodes % n_cores == 0
        self.n = n_nodes
        self.in_ch = in_ch
        self.hid = hid_ch
        self.cores = n_cores
        self.nsh = n_nodes // n_cores  # nodes per core
        self.nw = math.ceil(self.nsh / P)  # 128-node windows per core
        self.chunk = chunk  # gather chunk rows (int16 idx range)
        self.nchunk = math.ceil(n_nodes / chunk)
        self.sww = sww  # windows per super-window (PSUM tile width)
        self.nsw = math.ceil(self.nw / sww)
        assert self.nsh % self.nchunk == 0
        self.qsh = self.nsh // self.nchunk  # quarter-shard rows per core


CFG = Cfg(N_NODES, IN_CH, HID_CH, N_CORES, chunk=25000, sww=2)


def route(edge_index, cfg):
    """Host-side edge routing. Returns (dinv, per_core_arrays, structure).

    Edges are grouped per (super-window, chunk); each 128-edge block may mix
    destinations from any window of its super-window (S matrices span the
    whole sww*128-wide PSUM tile). Group sizes are padded to the max over
    cores (SPMD) rounded up to 128.
    """
    src = np.asarray(edge_index[0]).astype(np.int64)
    dst = np.asarray(edge_index[1]).astype(np.int64)

    deg = np.bincount(dst, minlength=cfg.n).astype(np.float64) + 1.0
    dinv = (1.0 / np.sqrt(deg)).astype(np.float32)

    ngrp = cfg.nsw * cfg.nchunk
    core = dst // cfg.nsh
    per_core = []
    counts = np.zeros((cfg.cores, ngrp), dtype=np.int64)
    for c in range(cfg.cores):
        m = core == c
        s = src[m]
        d = dst[m] - c * cfg.nsh
        sw = (d >> 7) // cfg.sww
        # xs_full layout is quarter-interleaved: node v (core sc, offset o,
        # quarter q = o // qsh, within = o % qsh) lives in chunk q at row
        # sc * qsh + within, so gathers for chunk q depend only on the
        # AllGather of every core's quarter q.
        sc = s // cfg.nsh
        o = s % cfg.nsh
        ch = o // cfg.qsh
        g = sw * cfg.nchunk + ch
        counts[c] = np.bincount(g, minlength=ngrp)
        per_core.append((s, d, g, (sc * cfg.qsh + o % cfg.qsh).astype(np.int64)))

    padded = np.ceil(np.max(counts, axis=0) / P).astype(np.int64) * P  # [ngrp]
    # stream order (p8 = group of super-windows, chunk, quad): one gather call
    # covers all quads of a p8 for one chunk
    spc = max(1, 8 // cfg.sww)  # super-windows per gather call (8 windows)
    pairs = (cfg.nsw + spc - 1) // spc
    order = []
    for p8 in range(pairs):
        for ch in range(cfg.nchunk):
            for q in range(spc * p8, min(spc * (p8 + 1), cfg.nsw)):
                order.append(q * cfg.nchunk + ch)
    order = np.array(order, dtype=np.int64)
    sizes = padded[order]
    st2 = np.zeros_like(sizes)
    st2[1:] = np.cumsum(sizes)[:-1]
    starts = np.zeros(ngrp, dtype=np.int64)
    starts[order] = st2
    total = int(padded.sum())

    calls = []
    for p8 in range(pairs):
        for ch in range(cfg.nchunk):
            gs = [
                q * cfg.nchunk + ch
                for q in range(spc * p8, min(spc * (p8 + 1), cfg.nsw))
            ]
            n = int(sum(padded[g] for g in gs))
            if n > 0:
                calls.append(
                    dict(
                        p8=p8,
                        ch=ch,
                        n=n,
                        off=int(starts[gs[0]]),
                        qnb=[int(padded[g]) // P for g in gs],
                    )
                )

    core_arrays = []
    for c in range(cfg.cores):
        s, d, g, row_in_chunk = per_core[c]
        idx = np.zeros(total, dtype=np.int16)
        rel = np.full(total, -1.0, dtype=np.float32)
        ordr = np.argsort(g, kind="stable")
        gs = g[ordr]
        grp_first = np.searchsorted(gs, np.arange(ngrp), "left")
        rank = np.arange(len(gs)) - grp_first[gs]
        slot = starts[gs] + rank
        sw_sorted = gs // cfg.nchunk
        idx[slot] = row_in_chunk[ordr].astype(np.int16)
        rel[slot] = (d[ordr] - sw_sorted * cfg.sww * P).astype(np.float32)
        idx_w = np.tile(idx.reshape(total // 16, 16).T, (8, 1)).copy()
        rel_w = rel.reshape(total // P, P).T.copy()
        core_arrays.append((idx_w, rel_w))

    return dinv, core_arrays, dict(calls=calls, total=total)


def build(cfg, structure):
    import concourse.bass as bass
    import concourse.tile as tile
    from concourse import bacc, mybir

    f32 = mybir.dt.float32
    bf16 = mybir.dt.float16  # fp16: exact small ints, 2x DVE mode
    i16 = mybir.dt.int16
    i32 = mybir.dt.int32
    AF = mybir.ActivationFunctionType
    OP = mybir.AluOpType

    calls = structure["calls"]
    total = structure["total"]

    nsh, nw, hid, in_ch = cfg.nsh, cfg.nw, cfg.hid, cfg.in_ch
    last_rows = nsh - (nw - 1) * P  # rows in the final (partial) window

    nc = bacc.Bacc(
        "TRN2", target_bir_lowering=False, debug=False, num_devices=cfg.cores,
        num_swdge_queues=4,
    )

    x_sh = nc.declare_dram_parameter("x_sh", [nsh, in_ch], f32, isOutput=False)
    dinv_t = nc.declare_dram_parameter("dinv_t", [P, nw], f32, isOutput=False)
    sqdeg_r = nc.declare_dram_parameter("sqdeg_r", [1, nw * P], bf16, isOutput=False)
    w_p = nc.declare_dram_parameter("w_p", [in_ch, hid], f32, isOutput=False)
    b_p = nc.declare_dram_parameter("b_p", [1, hid], f32, isOutput=False)
    alpha_p = nc.declare_dram_parameter("alpha_p", [1, 1], f32, isOutput=False)
    idx_p = nc.declare_dram_parameter("idx_p", [P, total // 16], i16, isOutput=False)
    rel_p = nc.declare_dram_parameter("rel_p", [P, total // P], f32, isOutput=False)
    out_p = nc.declare_dram_parameter("out", [nsh, hid], f32, isOutput=True)

    xs_loc = nc.dram_tensor("xs_loc", [nsh, in_ch], bf16)
    xs_full = nc.dram_tensor("xs_full", [cfg.n, in_ch], bf16, addr_space="Shared")

    with tile.TileContext(nc) as tc:
        with (
            tc.tile_pool(name="const", bufs=1) as constp,
            tc.tile_pool(name="xsbuf", bufs=1) as xsp,
            tc.tile_pool(name="xin", bufs=3) as xinp,
            tc.tile_pool(name="gath", bufs=20) as gp,
            tc.tile_pool(name="smat", bufs=16) as sp,
            tc.tile_pool(name="idxs", bufs=16) as idxp,
            tc.tile_pool(name="aggt", bufs=6) as aggp,
            tc.tile_pool(name="epi", bufs=4) as epip,
            tc.tile_pool(name="sqw", bufs=3) as sqwp,
            tc.tile_pool(name="psw", bufs=5, space="PSUM") as pswp,
            tc.tile_pool(name="pso", bufs=2, space="PSUM") as psop,
            tc.tile_pool(name="psa", bufs=1, space="PSUM") as psap,
        ):
            # ---- constants / setup ----
            wbc = cfg.sww * P
            iota_i = constp.tile([P, wbc], i32)
            nc.gpsimd.iota(iota_i[:], pattern=[[1, wbc]], base=0, channel_multiplier=0)
            iota_f = constp.tile([P, wbc], mybir.dt.float16)
            nc.vector.tensor_copy(iota_f[:], iota_i[:])
            lane_i = constp.tile([P, 1], i32)
            nc.gpsimd.iota(lane_i[:], pattern=[[1, 1]], base=0, channel_multiplier=1)
            lane_f = constp.tile([P, 1], f32)
            nc.vector.tensor_copy(lane_f[:], lane_i[:])
            ident_bf = constp.tile([P, P], bf16)
            nc.vector.tensor_scalar(
                ident_bf[:], iota_f[:, :P], lane_f[:], None, OP.is_equal
            )

            w_f32 = constp.tile([in_ch, hid], f32)
            nc.sync.dma_start(w_f32[:], w_p[:])
            w_bf = constp.tile([in_ch, hid], bf16)
            nc.vector.tensor_copy(w_bf[:], w_f32[:])

            b_f32 = constp.tile([1, hid], f32)
            nc.sync.dma_start(b_f32[:], b_p[:])
            b_bf = constp.tile([1, hid], bf16)
            nc.vector.tensor_copy(b_bf[:], b_f32[:])

            ones1 = constp.tile([1, P], f32)
            nc.vector.memset(ones1[:], 1.0)
            alpha_sb = constp.tile([1, 1], f32)
            nc.sync.dma_start(alpha_sb[:], alpha_p[:])

            dinv_sb = constp.tile([P, nw], f32)
            nc.sync.dma_start(dinv_sb[:], dinv_t[:])

            # alpha broadcast to [128,1] via K=1 matmul with ones
            psum_a = psap.tile([P, 1], f32, space="PSUM")
            nc.tensor.matmul(
                psum_a[:], lhsT=ones1[:], rhs=alpha_sb[:], start=True, stop=True
            )
            alpha_bc = constp.tile([P, 1], f32)
            nc.vector.tensor_copy(alpha_bc[:], psum_a[:])

            # ---- phase 1: xs = dinv * x ; AllGather ----
            xs_sb = xsp.tile([P, nw, in_ch], bf16)
            WG = 4
            nfull = (nw - 1) if last_rows < P else nw  # full 128-row windows
            wg = 0
            while wg < nfull - (nfull % WG):
                g = WG
                xt = xinp.tile([P, g, in_ch], f32, tag="xt")
                nc.scalar.dma_start(
                    xt[:],
                    x_sh[wg * P : (wg + g) * P, :].rearrange(
                        "(g p) c -> p g c", p=P
                    ),
                )
                for j in range(g):
                    nc.vector.tensor_scalar(
                        xs_sb[:, wg + j, :],
                        xt[:, j, :],
                        dinv_sb[:, wg + j : wg + j + 1],
                        None,
                        OP.mult,
                    )
                nc.scalar.dma_start(
                    xs_loc[wg * P : (wg + g) * P, :].rearrange(
                        "(g p) c -> p g c", p=P
                    ),
                    xs_sb[:, wg : wg + g, :],
                )
                wg += g
            for w in range(wg, nw):
                rows = P if w < nw - 1 else last_rows
                xt = xinp.tile([P, 1, in_ch], f32, tag="xt")
                if rows < P:
                    nc.vector.memset(xt[:], 0.0)
                nc.scalar.dma_start(xt[:rows, 0, :], x_sh[w * P : w * P + rows, :])
                nc.vector.tensor_scalar(
                    xs_sb[:, w, :], xt[:, 0, :], dinv_sb[:, w : w + 1], None, OP.mult
                )
                nc.scalar.dma_start(
                    xs_loc[w * P : w * P + rows, :], xs_sb[:rows, w, :]
                )

            # quartered AllGather: chunk q of xs_full = concat over cores of
            # each core's quarter q; gathers for chunk q wait only on AG_q
            for q in range(cfg.nchunk):
                nc.gpsimd.collective_compute(
                    "AllGather",
                    mybir.AluOpType.bypass,
                    replica_groups=[list(range(cfg.cores))],
                    ins=[xs_loc[q * cfg.qsh : (q + 1) * cfg.qsh, :]],
                    outs=[xs_full[q * cfg.chunk : (q + 1) * cfg.chunk, :]],
                )

            # ---- phase 2: gather + aggregate + epilogue ----
            wb = cfg.sww * P  # S width (<= one PSUM tile of dst cols)
            spc = max(1, 8 // cfg.sww)
            pairs = (cfg.nsw + spc - 1) // spc
            calls_by_p8 = {}
            for cl in calls:
                calls_by_p8.setdefault(cl["p8"], []).append(cl)

            def emit_epilogue(psw, ws):
                for wi, w in enumerate(ws):
                    rows = P if w < nw - 1 else last_rows
                    aggt = aggp.tile([P, P], bf16, tag="aggt")
                    nc.vector.tensor_copy(aggt[:], psw[:, wi * P : (wi + 1) * P])
                    sqw = sqwp.tile([1, P], bf16, tag="sq")
                    nc.sync.dma_start(sqw[:], sqdeg_r[:, w * P : (w + 1) * P])
                    pso = psop.tile([P, hid], f32, space="PSUM", tag="pso")
                    nc.tensor.matmul(
                        pso[:], lhsT=sqw[:], rhs=b_bf[:],
                        start=True, stop=False,
                    )
                    nc.tensor.matmul(
                        pso[:], lhsT=aggt[:], rhs=w_bf[:], start=False, stop=True
                    )
                    ot = epip.tile([P, hid], f32, tag="ot")
                    nc.scalar.activation(
                        ot[:], pso[:], AF.Prelu,
                        bias=0.0, scale=dinv_sb[:, w : w + 1], alpha=alpha_bc[:, :1],
                    )
                    nc.scalar.dma_start(out_p[w * P : w * P + rows, :], ot[:rows, :])

            for p8 in range(pairs):
                p8_calls = calls_by_p8.get(p8, [])
                # Split each (p8, chunk) call into two half-calls (quads 0-1 /
                # quads 2-3) and dispatch round-robin over the 4 SWDGE queues
                # (queue = chunk): each queue's descriptors are emitted by a
                # different Q7 pair, queue 0 blocks the Pool NX and paces
                # dispatch while 1-3 emit asynchronously; halving the calls
                # halves the convoy wait when a pair is still busy.
                halves = []  # (cl, half, off, n, qnb_half)
                for cl in p8_calls:
                    qnb = cl["qnb"]
                    for h in (0, 1):
                        qh = qnb[2 * h : 2 * h + 2]
                        nh = sum(qh) * P
                        if nh == 0:
                            continue
                        offh = cl["off"] + sum(qnb[: 2 * h]) * P
                        halves.append((cl, h, offh, nh, qh))
                halves.sort(key=lambda t: (t[1], t[0]["ch"] == 0, t[0]["ch"]))
                seg_by_ch = {}
                for cl, h, offh, nh, qh in halves:
                    nb = nh // P
                    it = idxp.tile([P, nh // 16], i16, tag="idx")
                    nc.sync.dma_start(
                        it[:], idx_p[:, offh // 16 : (offh + nh) // 16]
                    )
                    gt = gp.tile([P, nb, in_ch], bf16, tag="g")
                    ch0 = cl["ch"] * cfg.chunk
                    ch1 = min(ch0 + cfg.chunk, cfg.n)
                    nc.gpsimd.dma_gather(
                        gt[:], xs_full[ch0:ch1, :], it[:], nh, nh, in_ch,
                        single_packet=False, queue_num=cl["ch"],
                    )
                    rel_sb = idxp.tile([P, nb], f32, tag="rel")
                    nc.sync.dma_start(
                        rel_sb[:], rel_p[:, offh // P : offh // P + nb]
                    )
                    seg_by_ch.setdefault(cl["ch"], {})[h] = (gt, rel_sb, qh)

                nquad = min(spc, cfg.nsw - spc * p8)
                disp_order = [c for c in (1, 2, 3, 0) if c in seg_by_ch]
                for qi in range(nquad):
                    sw = spc * p8 + qi
                    ws = list(range(sw * cfg.sww, min((sw + 1) * cfg.sww, nw)))
                    h, hq = divmod(qi, 2)
                    # consume chunks in gather-landing order (queue 0 lands
                    # last: its dispatch is the blocking one)
                    qseg = [seg_by_ch[c][h] for c in disp_order if h in seg_by_ch[c]]
                    nmm = sum(
                        qh[hq] if hq < len(qh) else 0 for *_, qh in qseg
                    )
                    psw = pswp.tile([P, wb], f32, space="PSUM", tag="psw")
                    # self-loop injection; only the FIRST matmul of the tile
                    # sets start=True (zero_accum zeroes the whole tile)
                    for wi, w in enumerate(ws):
                        nc.tensor.matmul(
                            psw[:, wi * P : (wi + 1) * P],
                            lhsT=xs_sb[:, w, :],
                            rhs=ident_bf[:],
                            start=(wi == 0),
                            stop=(nmm == 0 and wi == len(ws) - 1),
                            skip_group_check=True,
                        )
                    wsw = len(ws) * P
                    k = 0
                    for gt, rel_sb, qh in qseg:
                        qoff = sum(qh[:hq])
                        for bi in range(qoff, qoff + (qh[hq] if hq < len(qh) else 0)):
                            st = sp.tile([P, wsw], bf16, tag="s")
                            nc.vector.tensor_scalar(
                                st[:],
                                iota_f[:, :wsw],
                                rel_sb[:, bi : bi + 1],
                                None,
                                OP.is_equal,
                            )
                            k += 1
                            nc.tensor.matmul(
                                psw[:, :wsw],
                                lhsT=gt[:, bi, :],
                                rhs=st[:],
                                start=False,
                                stop=(k == nmm),
                                skip_group_check=True,
                            )
                    emit_epilogue(psw, ws)

    nc.compile()
    return nc


def _prep_inputs(x, edge_index, W, b, alpha, cfg):
    dinv, core_arrays, structure = route(edge_index, cfg)
    x = np.asarray(x, dtype=np.float32)
    W = np.asarray(W, dtype=np.float32)
    b = np.asarray(b, dtype=np.float32).reshape(1, cfg.hid)
    alpha = np.asarray(alpha, dtype=np.float32).reshape(1, 1)

    pad_n = cfg.nw * P - cfg.nsh
    in_maps = []
    for c in range(cfg.cores):
        idx_w, rel_w = core_arrays[c]
        dsh = dinv[c * cfg.nsh : (c + 1) * cfg.nsh]
        dsh = np.concatenate([dsh, np.ones(pad_n, np.float32)])
        in_maps.append(
            {
                "x_sh": x[c * cfg.nsh : (c + 1) * cfg.nsh],
                "dinv_t": dsh.reshape(cfg.nw, P).T.copy(),
                "sqdeg_r": (1.0 / dsh).reshape(1, cfg.nw * P).astype(np.float16),
                "w_p": W,
                "b_p": b,
                "alpha_p": alpha,
                "idx_p": idx_w,
                "rel_p": rel_w,
            }
        )
    return in_maps, structure


def kernel(x, edge_index, W, b, alpha):
    from concourse.bass_utils import run_bass_kernel_spmd

    cfg = CFG
    in_maps, structure = _prep_inputs(x, edge_index, W, b, alpha, cfg)
    nc = build(cfg, structure)
    res = run_bass_kernel_spmd(nc, in_maps, list(range(cfg.cores)))
    out = np.concatenate(
        [np.asarray(res.results[c]["out"]) for c in range(cfg.cores)], axis=0
    )
    return out.astype(np.float32)


# revision 13
# speedup vs baseline: 1.4415x; 1.0867x over previous
"""GCNConv (aggregate in in_ch, then @W) + PReLU, distributed over 8 TRN2 NeuronCores.

Decomposition (matches the reference exactly):
    deg[v]  = in-degree of v including self-loop
    dinv    = deg ** -0.5
    xs[u]   = dinv[u] * x[u]                     (per-node src scale, fp16)
    rawagg[v] = sum_{e: dst=v} xs[src_e] + xs[v]
    out     = PReLU(dinv[v] * (rawagg @ W) + b)  (dinv[dst] applied in epilogue)

Sharding: nodes split contiguously over 8 cores (dst ownership); edges routed
host-side to the core owning their destination. Each core:
  1. scales its x shard -> xs shard (fp16), AllGather -> full xs in local DRAM
  2. dma_gather's xs[src] for its ~200k edges (int16 idxs into 25k-row chunk
     views of xs). Gather calls are spread round-robin over the 4 SWDGE
     queues: each queue's descriptors are emitted by a different GpSimd Q7
     core pair (ucode dispatches on cpu_id/2 == queue_num), and queues 1-3
     complete asynchronously at the NX, so 4 emissions run concurrently
     (~4x the single-queue descriptor rate, which is the kernel bottleneck).
  3. S[e, d] = [d == dst_rel_e] one-hot built on DVE (fp16 iota is_equal);
     TensorEngine contracts 128-edge blocks: psum[ch, dst] += G_blk^T @ S_blk
     accumulating one super-window; self-loops enter via xs_win^T @ I.
  4. per-window epilogue: pso = sqrtdeg^T b + aggT^T @ W (PSUM); single ACT
     Prelu op applies out = PReLU(dinv[dst]*pso) with per-partition scale and
     the runtime alpha; DMA out the [12500, 512] shard.
SPMD requires an identical instruction stream on all cores, so per-(sw,chunk)
group sizes are padded to the max over cores, rounded to 128 (pad edges gather
row 0 with dst_rel=-1 so their S row is all zero).
"""

import math

import numpy as np

# Problem constants (hardcoded per the task spec).
N_NODES = 100000
N_EDGES = 1600000
IN_CH = 128
HID_CH = 512
N_CORES = 8

P = 128  # partitions / window size


class Cfg:
    def __init__(self, n_nodes, in_ch, hid_ch, n_cores, chunk, sww):
        assert n_nodes % n_cores == 0
        self.n = n_nodes
        self.in_ch = in_ch
        self.hid = hid_ch
        self.cores = n_cores
        self.nsh = n_nodes // n_cores  # nodes per core
        self.nw = math.ceil(self.nsh / P)  # 128-node windows per core
        self.chunk = chunk  # gather chunk rows (int16 idx range)
        self.nchunk = math.ceil(n_nodes / chunk)
        self.sww = sww  # windows per super-window (PSUM tile width)
        self.nsw = math.ceil(self.nw / sww)
        assert self.nsh % self.nchunk == 0
        self.qsh = self.nsh // self.nchunk  # quarter-shard rows per core


CFG = Cfg(N_NODES, IN_CH, HID_CH, N_CORES, chunk=25000, sww=2)


def route(edge_index, cfg):
    """Host-side edge routing. Returns (dinv, per_core_arrays, structure).

    Edges are grouped per (super-window, chunk); each 128-edge block may mix
    destinations from any window of its super-window (S matrices span the
    whole sww*128-wide PSUM tile). Group sizes are padded to the max over
    cores (SPMD) rounded up to 128.
    """
    src = np.asarray(edge_index[0]).astype(np.int64)
    dst = np.asarray(edge_index[1]).astype(np.int64)

    deg = np.bincount(dst, minlength=cfg.n).astype(np.float64) + 1.0
    dinv = (1.0 / np.sqrt(deg)).astype(np.float32)

    ngrp = cfg.nsw * cfg.nchunk
    core = dst // cfg.nsh
    per_core = []
    counts = np.zeros((cfg.cores, ngrp), dtype=np.int64)
    for c in range(cfg.cores):
        m = core == c
        s = src[m]
        d = dst[m] - c * cfg.nsh
        sw = (d >> 7) // cfg.sww
        # xs_full layout is quarter-interleaved: node v (core sc, offset o,
        # quarter q = o // qsh, within = o % qsh) lives in chunk q at row
        # sc * qsh + within, so gathers for chunk q depend only on the
        # AllGather of every core's quarter q.
        sc = s // cfg.nsh
        o = s % cfg.nsh
        ch = o // cfg.qsh
        g = sw * cfg.nchunk + ch
        counts[c] = np.bincount(g, minlength=ngrp)
        per_core.append((s, d, g, (sc * cfg.qsh + o % cfg.qsh).astype(np.int64)))

    padded = np.ceil(np.max(counts, axis=0) / P).astype(np.int64) * P  # [ngrp]
    # stream order (p8 = group of super-windows, chunk, quad): one gather call
    # covers all quads of a p8 for one chunk
    spc = max(1, 8 // cfg.sww)  # super-windows per gather call (8 windows)
    pairs = (cfg.nsw + spc - 1) // spc
    order = []
    for p8 in range(pairs):
        for ch in range(cfg.nchunk):
            for q in range(spc * p8, min(spc * (p8 + 1), cfg.nsw)):
                order.append(q * cfg.nchunk + ch)
    order = np.array(order, dtype=np.int64)
    sizes = padded[order]
    st2 = np.zeros_like(sizes)
    st2[1:] = np.cumsum(sizes)[:-1]
    starts = np.zeros(ngrp, dtype=np.int64)
    starts[order] = st2
    total = int(padded.sum())

    calls = []
    for p8 in range(pairs):
        for ch in range(cfg.nchunk):
            gs = [
                q * cfg.nchunk + ch
                for q in range(spc * p8, min(spc * (p8 + 1), cfg.nsw))
            ]
            n = int(sum(padded[g] for g in gs))
            if n > 0:
                calls.append(
                    dict(
                        p8=p8,
                        ch=ch,
                        n=n,
                        off=int(starts[gs[0]]),
                        qnb=[int(padded[g]) // P for g in gs],
                    )
                )

    core_arrays = []
    for c in range(cfg.cores):
        s, d, g, row_in_chunk = per_core[c]
        idx = np.zeros(total, dtype=np.int16)
        rel = np.full(total, -1.0, dtype=np.float32)
        ordr = np.argsort(g, kind="stable")
        gs = g[ordr]
        grp_first = np.searchsorted(gs, np.arange(ngrp), "left")
        rank = np.arange(len(gs)) - grp_first[gs]
        slot = starts[gs] + rank
        sw_sorted = gs // cfg.nchunk
        idx[slot] = row_in_chunk[ordr].astype(np.int16)
        rel[slot] = (d[ordr] - sw_sorted * cfg.sww * P).astype(np.float32)
        idx_w = np.tile(idx.reshape(total // 16, 16).T, (8, 1)).copy()
        rel_w = rel.reshape(total // P, P).T.copy()
        core_arrays.append((idx_w, rel_w))

    return dinv, core_arrays, dict(calls=calls, total=total)


def build(cfg, structure):
    import concourse.bass as bass
    import concourse.tile as tile
    from concourse import bacc, mybir

    f32 = mybir.dt.float32
    bf16 = mybir.dt.float16  # fp16: exact small ints, 2x DVE mode
    i16 = mybir.dt.int16
    i32 = mybir.dt.int32
    AF = mybir.ActivationFunctionType
    OP = mybir.AluOpType

    calls = structure["calls"]
    total = structure["total"]

    nsh, nw, hid, in_ch = cfg.nsh, cfg.nw, cfg.hid, cfg.in_ch
    last_rows = nsh - (nw - 1) * P  # rows in the final (partial) window

    nc = bacc.Bacc(
        "TRN2", target_bir_lowering=False, debug=False, num_devices=cfg.cores,
        num_swdge_queues=4,
    )

    x_sh = nc.declare_dram_parameter("x_sh", [nsh, in_ch], f32, isOutput=False)
    dinv_t = nc.declare_dram_parameter("dinv_t", [P, nw], f32, isOutput=False)
    sqdeg_r = nc.declare_dram_parameter("sqdeg_r", [1, nw * P], bf16, isOutput=False)
    w_p = nc.declare_dram_parameter("w_p", [in_ch, hid], f32, isOutput=False)
    b_p = nc.declare_dram_parameter("b_p", [1, hid], f32, isOutput=False)
    alpha_p = nc.declare_dram_parameter("alpha_p", [1, 1], f32, isOutput=False)
    idx_p = nc.declare_dram_parameter("idx_p", [P, total // 16], i16, isOutput=False)
    rel_p = nc.declare_dram_parameter("rel_p", [P, total // P], f32, isOutput=False)
    out_p = nc.declare_dram_parameter("out", [nsh, hid], f32, isOutput=True)

    xs_loc = nc.dram_tensor("xs_loc", [nsh, in_ch], bf16)
    xs_full = nc.dram_tensor("xs_full", [cfg.n, in_ch], bf16, addr_space="Shared")

    with tile.TileContext(nc) as tc:
        with (
            tc.tile_pool(name="const", bufs=1) as constp,
            tc.tile_pool(name="xsbuf", bufs=1) as xsp,
            tc.tile_pool(name="xin", bufs=2) as xinp,
            tc.tile_pool(name="gath", bufs=30) as gp,
            tc.tile_pool(name="smat", bufs=8) as sp,
            tc.tile_pool(name="idxs", bufs=12) as idxp,
            tc.tile_pool(name="aggt", bufs=6) as aggp,
            tc.tile_pool(name="epi", bufs=3) as epip,
            tc.tile_pool(name="sqw", bufs=3) as sqwp,
            tc.tile_pool(name="psw", bufs=6, space="PSUM") as pswp,
            tc.tile_pool(name="pso", bufs=2, space="PSUM") as psop,
        ):
            # ---- constants / setup ----
            wbc = cfg.sww * P
            iota_i = constp.tile([P, wbc], i32)
            nc.gpsimd.iota(iota_i[:], pattern=[[1, wbc]], base=0, channel_multiplier=0)
            iota_f = constp.tile([P, wbc], mybir.dt.float16)
            nc.vector.tensor_copy(iota_f[:], iota_i[:])
            lane_i = constp.tile([P, 1], i32)
            nc.gpsimd.iota(lane_i[:], pattern=[[1, 1]], base=0, channel_multiplier=1)
            lane_f = constp.tile([P, 1], f32)
            nc.vector.tensor_copy(lane_f[:], lane_i[:])
            ident_bf = constp.tile([P, P], bf16)
            nc.vector.tensor_scalar(
                ident_bf[:], iota_f[:, :P], lane_f[:], None, OP.is_equal
            )

            w_f32 = constp.tile([in_ch, hid], f32)
            nc.sync.dma_start(w_f32[:], w_p[:])
            w_bf = constp.tile([in_ch, hid], bf16)
            nc.vector.tensor_copy(w_bf[:], w_f32[:])

            b_f32 = constp.tile([1, hid], f32)
            nc.sync.dma_start(b_f32[:], b_p[:])
            b_bf = constp.tile([1, hid], bf16)
            nc.vector.tensor_copy(b_bf[:], b_f32[:])

            ones1 = constp.tile([1, P], f32)
            nc.vector.memset(ones1[:], 1.0)
            alpha_sb = constp.tile([1, 1], f32)
            nc.sync.dma_start(alpha_sb[:], alpha_p[:])

            dinv_sb = constp.tile([P, nw], f32)
            nc.sync.dma_start(dinv_sb[:], dinv_t[:])

            # alpha broadcast to [128,1] via K=1 matmul with ones
            psum_a = pswp.tile([P, cfg.sww * P], f32, space="PSUM", tag="psw")
            nc.tensor.matmul(
                psum_a[:, :1], lhsT=ones1[:], rhs=alpha_sb[:], start=True, stop=True
            )
            alpha_bc = constp.tile([P, 1], f32)
            nc.vector.tensor_copy(alpha_bc[:], psum_a[:, :1])

            # ---- phase 1: xs = dinv * x ; AllGather ----
            xs_sb = xsp.tile([P, nw, in_ch], bf16)
            WG = 4
            nfull = (nw - 1) if last_rows < P else nw  # full 128-row windows
            wg = 0
            while wg < nfull - (nfull % WG):
                g = WG
                xt = xinp.tile([P, g, in_ch], f32, tag="xt")
                nc.scalar.dma_start(
                    xt[:],
                    x_sh[wg * P : (wg + g) * P, :].rearrange(
                        "(g p) c -> p g c", p=P
                    ),
                )
                for j in range(g):
                    nc.vector.tensor_scalar(
                        xs_sb[:, wg + j, :],
                        xt[:, j, :],
                        dinv_sb[:, wg + j : wg + j + 1],
                        None,
                        OP.mult,
                    )
                nc.scalar.dma_start(
                    xs_loc[wg * P : (wg + g) * P, :].rearrange(
                        "(g p) c -> p g c", p=P
                    ),
                    xs_sb[:, wg : wg + g, :],
                )
                wg += g
            for w in range(wg, nw):
                rows = P if w < nw - 1 else last_rows
                xt = xinp.tile([P, 1, in_ch], f32, tag="xt")
                if rows < P:
                    nc.vector.memset(xt[:], 0.0)
                nc.scalar.dma_start(xt[:rows, 0, :], x_sh[w * P : w * P + rows, :])
                nc.vector.tensor_scalar(
                    xs_sb[:, w, :], xt[:, 0, :], dinv_sb[:, w : w + 1], None, OP.mult
                )
                nc.scalar.dma_start(
                    xs_loc[w * P : w * P + rows, :], xs_sb[:rows, w, :]
                )

            # quartered AllGather: chunk q of xs_full = concat over cores of
            # each core's quarter q; gathers for chunk q wait only on AG_q
            for q in range(cfg.nchunk):
                nc.gpsimd.collective_compute(
                    "AllGather",
                    mybir.AluOpType.bypass,
                    replica_groups=[list(range(cfg.cores))],
                    ins=[xs_loc[q * cfg.qsh : (q + 1) * cfg.qsh, :]],
                    outs=[xs_full[q * cfg.chunk : (q + 1) * cfg.chunk, :]],
                )

            # ---- phase 2: gather + aggregate + epilogue ----
            wb = cfg.sww * P  # S width (<= one PSUM tile of dst cols)
            spc = max(1, 8 // cfg.sww)
            pairs = (cfg.nsw + spc - 1) // spc
            calls_by_p8 = {}
            for cl in calls:
                calls_by_p8.setdefault(cl["p8"], []).append(cl)

            def emit_epilogue(psw, ws):
                for wi, w in enumerate(ws):
                    rows = P if w < nw - 1 else last_rows
                    aggt = aggp.tile([P, P], bf16, tag="aggt")
                    nc.vector.tensor_copy(aggt[:], psw[:, wi * P : (wi + 1) * P])
                    sqw = sqwp.tile([1, P], bf16, tag="sq")
                    nc.sync.dma_start(sqw[:], sqdeg_r[:, w * P : (w + 1) * P])
                    pso = psop.tile([P, hid], f32, space="PSUM", tag="pso")
                    nc.tensor.matmul(
                        pso[:], lhsT=sqw[:], rhs=b_bf[:],
                        start=True, stop=False,
                    )
                    nc.tensor.matmul(
                        pso[:], lhsT=aggt[:], rhs=w_bf[:], start=False, stop=True
                    )
                    ot = epip.tile([P, hid], f32, tag="ot")
                    nc.scalar.activation(
                        ot[:], pso[:], AF.Prelu,
                        bias=0.0, scale=dinv_sb[:, w : w + 1], alpha=alpha_bc[:, :1],
                    )
                    nc.scalar.dma_start(out_p[w * P : w * P + rows, :], ot[:rows, :])

            for p8 in range(pairs):
                p8_calls = calls_by_p8.get(p8, [])
                # Split each (p8, chunk) call into two half-calls (quads 0-1 /
                # quads 2-3) and dispatch round-robin over the 4 SWDGE queues
                # (queue = chunk): each queue's descriptors are emitted by a
                # different Q7 pair, queue 0 blocks the Pool NX and paces
                # dispatch while 1-3 emit asynchronously; halving the calls
                # halves the convoy wait when a pair is still busy.
                halves = []  # (cl, half, off, n, qnb_half)
                for cl in p8_calls:
                    qnb = cl["qnb"]
                    for h in (0, 1):
                        qh = qnb[2 * h : 2 * h + 2]
                        nh = sum(qh) * P
                        if nh == 0:
                            continue
                        offh = cl["off"] + sum(qnb[: 2 * h]) * P
                        halves.append((cl, h, offh, nh, qh))
                halves.sort(key=lambda t: (t[1], t[0]["ch"] == 0, t[0]["ch"]))
                seg_by_ch = {}
                for cl, h, offh, nh, qh in halves:
                    nb = nh // P
                    it = idxp.tile([P, nh // 16], i16, tag="idx")
                    nc.sync.dma_start(
                        it[:], idx_p[:, offh // 16 : (offh + nh) // 16]
                    )
                    gt = gp.tile([P, nb, in_ch], bf16, tag="g")
                    ch0 = cl["ch"] * cfg.chunk
                    ch1 = min(ch0 + cfg.chunk, cfg.n)
                    nc.gpsimd.dma_gather(
                        gt[:], xs_full[ch0:ch1, :], it[:], nh, nh, in_ch,
                        single_packet=False, queue_num=cl["ch"],
                    )
                    rel_sb = idxp.tile([P, nb], f32, tag="rel")
                    nc.sync.dma_start(
                        rel_sb[:], rel_p[:, offh // P : offh // P + nb]
                    )
                    seg_by_ch.setdefault(cl["ch"], {})[h] = (gt, rel_sb, qh)

                nquad = min(spc, cfg.nsw - spc * p8)
                disp_order = [c for c in (1, 2, 3, 0) if c in seg_by_ch]
                for qi in range(nquad):
                    sw = spc * p8 + qi
                    ws = list(range(sw * cfg.sww, min((sw + 1) * cfg.sww, nw)))
                    h, hq = divmod(qi, 2)
                    # consume chunks in gather-landing order (queue 0 lands
                    # last: its dispatch is the blocking one)
                    qseg = [seg_by_ch[c][h] for c in disp_order if h in seg_by_ch[c]]
                    nmm = sum(
                        qh[hq] if hq < len(qh) else 0 for *_, qh in qseg
                    )
                    psw = pswp.tile([P, wb], f32, space="PSUM", tag="psw")
                    # self-loop injection; only the FIRST matmul of the tile
                    # sets start=True (zero_accum zeroes the whole tile)
                    for wi, w in enumerate(ws):
                        nc.tensor.matmul(
                            psw[:, wi * P : (wi + 1) * P],
                            lhsT=xs_sb[:, w, :],
                            rhs=ident_bf[:],
                            start=(wi == 0),
                            stop=(nmm == 0 and wi == len(ws) - 1),
                            skip_group_check=True,
                        )
                    wsw = len(ws) * P
                    k = 0
                    for gt, rel_sb, qh in qseg:
                        qoff = sum(qh[:hq])
                        for bi in range(qoff, qoff + (qh[hq] if hq < len(qh) else 0)):
                            st = sp.tile([P, wsw], bf16, tag="s")
                            nc.vector.tensor_scalar(
                                st[:],
                                iota_f[:, :wsw],
                                rel_sb[:, bi : bi + 1],
                                None,
                                OP.is_equal,
                            )
                            k += 1
                            nc.tensor.matmul(
                                psw[:, :wsw],
                                lhsT=gt[:, bi, :],
                                rhs=st[:],
                                start=False,
                                stop=(k == nmm),
                                skip_group_check=True,
                            )
                    emit_epilogue(psw, ws)

    nc.compile()
    return nc


def _prep_inputs(x, edge_index, W, b, alpha, cfg):
    dinv, core_arrays, structure = route(edge_index, cfg)
    x = np.asarray(x, dtype=np.float32)
    W = np.asarray(W, dtype=np.float32)
    b = np.asarray(b, dtype=np.float32).reshape(1, cfg.hid)
    alpha = np.asarray(alpha, dtype=np.float32).reshape(1, 1)

    pad_n = cfg.nw * P - cfg.nsh
    in_maps = []
    for c in range(cfg.cores):
        idx_w, rel_w = core_arrays[c]
        dsh = dinv[c * cfg.nsh : (c + 1) * cfg.nsh]
        dsh = np.concatenate([dsh, np.ones(pad_n, np.float32)])
        in_maps.append(
            {
                "x_sh": x[c * cfg.nsh : (c + 1) * cfg.nsh],
                "dinv_t": dsh.reshape(cfg.nw, P).T.copy(),
                "sqdeg_r": (1.0 / dsh).reshape(1, cfg.nw * P).astype(np.float16),
                "w_p": W,
                "b_p": b,
                "alpha_p": alpha,
                "idx_p": idx_w,
                "rel_p": rel_w,
            }
        )
    return in_maps, structure


def kernel(x, edge_index, W, b, alpha):
    from concourse.bass_utils import run_bass_kernel_spmd

    cfg = CFG
    in_maps, structure = _prep_inputs(x, edge_index, W, b, alpha, cfg)
    nc = build(cfg, structure)
    res = run_bass_kernel_spmd(nc, in_maps, list(range(cfg.cores)))
    out = np.concatenate(
        [np.asarray(res.results[c]["out"]) for c in range(cfg.cores)], axis=0
    )
    return out.astype(np.float32)


# revision 14
# speedup vs baseline: 1.4746x; 1.0229x over previous
"""GCNConv (aggregate in in_ch, then @W) + PReLU, distributed over 8 TRN2 NeuronCores.

Decomposition (matches the reference exactly):
    deg[v]  = in-degree of v including self-loop
    dinv    = deg ** -0.5
    xs[u]   = dinv[u] * x[u]                     (per-node src scale, fp16)
    rawagg[v] = sum_{e: dst=v} xs[src_e] + xs[v]
    out     = PReLU(dinv[v] * (rawagg @ W) + b)  (dinv[dst] applied in epilogue)

Sharding: nodes split contiguously over 8 cores (dst ownership); edges routed
host-side to the core owning their destination. Each core:
  1. scales its x shard -> xs shard (fp16), AllGather -> full xs in local DRAM
  2. dma_gather's xs[src] for its ~200k edges (int16 idxs into 25k-row chunk
     views of xs). Gather calls are spread round-robin over the 4 SWDGE
     queues: each queue's descriptors are emitted by a different GpSimd Q7
     core pair (ucode dispatches on cpu_id/2 == queue_num), and queues 1-3
     complete asynchronously at the NX, so 4 emissions run concurrently
     (~4x the single-queue descriptor rate, which is the kernel bottleneck).
  3. S[e, d] = [d == dst_rel_e] one-hot built on DVE (fp16 iota is_equal);
     TensorEngine contracts 128-edge blocks: psum[ch, dst] += G_blk^T @ S_blk
     accumulating one super-window; self-loops enter via xs_win^T @ I.
  4. per-window epilogue: pso = sqrtdeg^T b + aggT^T @ W (PSUM); single ACT
     Prelu op applies out = PReLU(dinv[dst]*pso) with per-partition scale and
     the runtime alpha; DMA out the [12500, 512] shard.
SPMD requires an identical instruction stream on all cores, so per-(sw,chunk)
group sizes are padded to the max over cores, rounded to 128 (pad edges gather
row 0 with dst_rel=-1 so their S row is all zero).
"""

import math

import numpy as np

# Problem constants (hardcoded per the task spec).
N_NODES = 100000
N_EDGES = 1600000
IN_CH = 128
HID_CH = 512
N_CORES = 8

P = 128  # partitions / window size


class Cfg:
    def __init__(self, n_nodes, in_ch, hid_ch, n_cores, chunk, sww):
        assert n_nodes % n_cores == 0
        self.n = n_nodes
        self.in_ch = in_ch
        self.hid = hid_ch
        self.cores = n_cores
        self.nsh = n_nodes // n_cores  # nodes per core
        self.nw = math.ceil(self.nsh / P)  # 128-node windows per core
        self.chunk = chunk  # gather chunk rows (int16 idx range)
        self.nchunk = math.ceil(n_nodes / chunk)
        self.sww = sww  # windows per super-window (PSUM tile width)
        self.nsw = math.ceil(self.nw / sww)
        assert self.nsh % self.nchunk == 0
        self.qsh = self.nsh // self.nchunk  # quarter-shard rows per core


CFG = Cfg(N_NODES, IN_CH, HID_CH, N_CORES, chunk=25000, sww=2)


def route(edge_index, cfg):
    """Host-side edge routing. Returns (dinv, per_core_arrays, structure).

    Edges are grouped per (super-window, chunk); each 128-edge block may mix
    destinations from any window of its super-window (S matrices span the
    whole sww*128-wide PSUM tile). Group sizes are padded to the max over
    cores (SPMD) rounded up to 128.
    """
    src = np.asarray(edge_index[0]).astype(np.int64)
    dst = np.asarray(edge_index[1]).astype(np.int64)

    deg = np.bincount(dst, minlength=cfg.n).astype(np.float64) + 1.0
    dinv = (1.0 / np.sqrt(deg)).astype(np.float32)

    ngrp = cfg.nsw * cfg.nchunk
    core = dst // cfg.nsh
    per_core = []
    counts = np.zeros((cfg.cores, ngrp), dtype=np.int64)
    for c in range(cfg.cores):
        m = core == c
        s = src[m]
        d = dst[m] - c * cfg.nsh
        sw = (d >> 7) // cfg.sww
        # xs_full layout is quarter-interleaved: node v (core sc, offset o,
        # quarter q = o // qsh, within = o % qsh) lives in chunk q at row
        # sc * qsh + within, so gathers for chunk q depend only on the
        # AllGather of every core's quarter q.
        sc = s // cfg.nsh
        o = s % cfg.nsh
        ch = o // cfg.qsh
        g = sw * cfg.nchunk + ch
        counts[c] = np.bincount(g, minlength=ngrp)
        per_core.append((s, d, g, (sc * cfg.qsh + o % cfg.qsh).astype(np.int64)))

    padded = np.ceil(np.max(counts, axis=0) / P).astype(np.int64) * P  # [ngrp]
    # stream order (p8 = group of super-windows, chunk, quad): one gather call
    # covers all quads of a p8 for one chunk
    spc = max(1, 8 // cfg.sww)  # super-windows per gather call (8 windows)
    pairs = (cfg.nsw + spc - 1) // spc
    order = []
    for p8 in range(pairs):
        for ch in range(cfg.nchunk):
            for q in range(spc * p8, min(spc * (p8 + 1), cfg.nsw)):
                order.append(q * cfg.nchunk + ch)
    order = np.array(order, dtype=np.int64)
    sizes = padded[order]
    st2 = np.zeros_like(sizes)
    st2[1:] = np.cumsum(sizes)[:-1]
    starts = np.zeros(ngrp, dtype=np.int64)
    starts[order] = st2
    total = int(padded.sum())

    calls = []
    for p8 in range(pairs):
        for ch in range(cfg.nchunk):
            gs = [
                q * cfg.nchunk + ch
                for q in range(spc * p8, min(spc * (p8 + 1), cfg.nsw))
            ]
            n = int(sum(padded[g] for g in gs))
            if n > 0:
                calls.append(
                    dict(
                        p8=p8,
                        ch=ch,
                        n=n,
                        off=int(starts[gs[0]]),
                        qnb=[int(padded[g]) // P for g in gs],
                    )
                )

    core_arrays = []
    for c in range(cfg.cores):
        s, d, g, row_in_chunk = per_core[c]
        idx = np.zeros(total, dtype=np.int16)
        rel = np.full(total, -1.0, dtype=np.float32)
        ordr = np.argsort(g, kind="stable")
        gs = g[ordr]
        grp_first = np.searchsorted(gs, np.arange(ngrp), "left")
        rank = np.arange(len(gs)) - grp_first[gs]
        slot = starts[gs] + rank
        sw_sorted = gs // cfg.nchunk
        idx[slot] = row_in_chunk[ordr].astype(np.int16)
        rel[slot] = (d[ordr] - sw_sorted * cfg.sww * P).astype(np.float32)
        idx_w = np.tile(idx.reshape(total // 16, 16).T, (8, 1)).copy()
        rel_w = rel.reshape(total // P, P).T.copy()
        core_arrays.append((idx_w, rel_w))

    return dinv, core_arrays, dict(calls=calls, total=total)


def build(cfg, structure):
    import concourse.bass as bass
    import concourse.tile as tile
    from concourse import bacc, mybir

    f32 = mybir.dt.float32
    bf16 = mybir.dt.float16  # fp16: exact small ints, 2x DVE mode
    i16 = mybir.dt.int16
    i32 = mybir.dt.int32
    AF = mybir.ActivationFunctionType
    OP = mybir.AluOpType

    calls = structure["calls"]
    total = structure["total"]

    nsh, nw, hid, in_ch = cfg.nsh, cfg.nw, cfg.hid, cfg.in_ch
    last_rows = nsh - (nw - 1) * P  # rows in the final (partial) window

    nc = bacc.Bacc(
        "TRN2", target_bir_lowering=False, debug=False, num_devices=cfg.cores,
        num_swdge_queues=4,
    )

    x_sh = nc.declare_dram_parameter("x_sh", [nsh, in_ch], f32, isOutput=False)
    dinv_t = nc.declare_dram_parameter("dinv_t", [P, nw], f32, isOutput=False)
    sqdeg_r = nc.declare_dram_parameter("sqdeg_r", [1, nw * P], bf16, isOutput=False)
    w_p = nc.declare_dram_parameter("w_p", [in_ch, hid], f32, isOutput=False)
    b_p = nc.declare_dram_parameter("b_p", [1, hid], f32, isOutput=False)
    alpha_p = nc.declare_dram_parameter("alpha_p", [1, 1], f32, isOutput=False)
    idx_p = nc.declare_dram_parameter("idx_p", [P, total // 16], i16, isOutput=False)
    rel_p = nc.declare_dram_parameter("rel_p", [P, total // P], f32, isOutput=False)
    out_p = nc.declare_dram_parameter("out", [nsh, hid], f32, isOutput=True)

    xs_loc = nc.dram_tensor("xs_loc", [nsh, in_ch], bf16)
    xs_full = nc.dram_tensor("xs_full", [cfg.n, in_ch], bf16, addr_space="Shared")

    with tile.TileContext(nc) as tc:
        with (
            tc.tile_pool(name="const", bufs=1) as constp,
            tc.tile_pool(name="xsbuf", bufs=1) as xsp,
            tc.tile_pool(name="xin", bufs=2) as xinp,
            tc.tile_pool(name="gath", bufs=30) as gp,
            tc.tile_pool(name="smat", bufs=8) as sp,
            tc.tile_pool(name="idxs", bufs=12) as idxp,
            tc.tile_pool(name="aggt", bufs=6) as aggp,
            tc.tile_pool(name="epi", bufs=3) as epip,
            tc.tile_pool(name="sqw", bufs=3) as sqwp,
            tc.tile_pool(name="psw", bufs=6, space="PSUM") as pswp,
            tc.tile_pool(name="pso", bufs=2, space="PSUM") as psop,
        ):
            # ---- constants / setup ----
            wbc = cfg.sww * P
            iota_i = constp.tile([P, wbc], i32)
            nc.gpsimd.iota(iota_i[:], pattern=[[1, wbc]], base=0, channel_multiplier=0)
            iota_f = constp.tile([P, wbc], mybir.dt.float16)
            nc.vector.tensor_copy(iota_f[:], iota_i[:])
            lane_i = constp.tile([P, 1], i32)
            nc.gpsimd.iota(lane_i[:], pattern=[[1, 1]], base=0, channel_multiplier=1)
            lane_f = constp.tile([P, 1], f32)
            nc.vector.tensor_copy(lane_f[:], lane_i[:])
            ident_bf = constp.tile([P, P], bf16)
            nc.vector.tensor_scalar(
                ident_bf[:], iota_f[:, :P], lane_f[:], None, OP.is_equal
            )

            w_f32 = constp.tile([in_ch, hid], f32)
            nc.sync.dma_start(w_f32[:], w_p[:])
            w_bf = constp.tile([in_ch, hid], bf16)
            nc.vector.tensor_copy(w_bf[:], w_f32[:])

            b_f32 = constp.tile([1, hid], f32)
            nc.sync.dma_start(b_f32[:], b_p[:])
            b_bf = constp.tile([1, hid], bf16)
            nc.vector.tensor_copy(b_bf[:], b_f32[:])

            ones1 = constp.tile([1, P], f32)
            nc.vector.memset(ones1[:], 1.0)
            alpha_sb = constp.tile([1, 1], f32)
            nc.sync.dma_start(alpha_sb[:], alpha_p[:])

            dinv_sb = constp.tile([P, nw], f32)
            nc.sync.dma_start(dinv_sb[:], dinv_t[:])

            # alpha broadcast to [128,1] via K=1 matmul with ones
            psum_a = pswp.tile([P, cfg.sww * P], f32, space="PSUM", tag="psw")
            nc.tensor.matmul(
                psum_a[:, :1], lhsT=ones1[:], rhs=alpha_sb[:], start=True, stop=True
            )
            alpha_bc = constp.tile([P, 1], f32)
            nc.vector.tensor_copy(alpha_bc[:], psum_a[:, :1])

            # ---- phase 1: xs = dinv * x ; AllGather ----
            xs_sb = xsp.tile([P, nw, in_ch], bf16)
            WG = 4
            nfull = (nw - 1) if last_rows < P else nw  # full 128-row windows
            wg = 0
            while wg < nfull - (nfull % WG):
                g = WG
                xt = xinp.tile([P, g, in_ch], f32, tag="xt")
                nc.scalar.dma_start(
                    xt[:],
                    x_sh[wg * P : (wg + g) * P, :].rearrange(
                        "(g p) c -> p g c", p=P
                    ),
                )
                for j in range(g):
                    nc.vector.tensor_scalar(
                        xs_sb[:, wg + j, :],
                        xt[:, j, :],
                        dinv_sb[:, wg + j : wg + j + 1],
                        None,
                        OP.mult,
                    )
                nc.scalar.dma_start(
                    xs_loc[wg * P : (wg + g) * P, :].rearrange(
                        "(g p) c -> p g c", p=P
                    ),
                    xs_sb[:, wg : wg + g, :],
                )
                wg += g
            for w in range(wg, nw):
                rows = P if w < nw - 1 else last_rows
                xt = xinp.tile([P, 1, in_ch], f32, tag="xt")
                if rows < P:
                    nc.vector.memset(xt[:], 0.0)
                nc.scalar.dma_start(xt[:rows, 0, :], x_sh[w * P : w * P + rows, :])
                nc.vector.tensor_scalar(
                    xs_sb[:, w, :], xt[:, 0, :], dinv_sb[:, w : w + 1], None, OP.mult
                )
                nc.scalar.dma_start(
                    xs_loc[w * P : w * P + rows, :], xs_sb[:rows, w, :]
                )

            # quartered AllGather: chunk q of xs_full = concat over cores of
            # each core's quarter q; gathers for chunk q wait only on AG_q
            for q in range(cfg.nchunk):
                nc.gpsimd.collective_compute(
                    "AllGather",
                    mybir.AluOpType.bypass,
                    replica_groups=[list(range(cfg.cores))],
                    ins=[xs_loc[q * cfg.qsh : (q + 1) * cfg.qsh, :]],
                    outs=[xs_full[q * cfg.chunk : (q + 1) * cfg.chunk, :]],
                )

            # ---- phase 2: gather + aggregate + epilogue ----
            wb = cfg.sww * P  # S width (<= one PSUM tile of dst cols)
            spc = max(1, 8 // cfg.sww)
            pairs = (cfg.nsw + spc - 1) // spc
            calls_by_p8 = {}
            for cl in calls:
                calls_by_p8.setdefault(cl["p8"], []).append(cl)

            def emit_epilogue(psw, ws):
                for wi, w in enumerate(ws):
                    rows = P if w < nw - 1 else last_rows
                    aggt = aggp.tile([P, P], bf16, tag="aggt")
                    nc.vector.tensor_copy(aggt[:], psw[:, wi * P : (wi + 1) * P])
                    sqw = sqwp.tile([1, P], bf16, tag="sq")
                    nc.sync.dma_start(sqw[:], sqdeg_r[:, w * P : (w + 1) * P])
                    pso = psop.tile([P, hid], f32, space="PSUM", tag="pso")
                    nc.tensor.matmul(
                        pso[:], lhsT=sqw[:], rhs=b_bf[:],
                        start=True, stop=False,
                    )
                    nc.tensor.matmul(
                        pso[:], lhsT=aggt[:], rhs=w_bf[:], start=False, stop=True
                    )
                    ot = epip.tile([P, hid], f32, tag="ot")
                    nc.scalar.activation(
                        ot[:], pso[:], AF.Prelu,
                        bias=0.0, scale=dinv_sb[:, w : w + 1], alpha=alpha_bc[:, :1],
                    )
                    nc.scalar.dma_start(out_p[w * P : w * P + rows, :], ot[:rows, :])

            for p8 in range(pairs):
                p8_calls = calls_by_p8.get(p8, [])
                # Split each (p8, chunk) call into two half-calls (quads 0-1 /
                # quads 2-3) and dispatch round-robin over the 4 SWDGE queues
                # (queue = chunk): each queue's descriptors are emitted by a
                # different Q7 pair, queue 0 blocks the Pool NX and paces
                # dispatch while 1-3 emit asynchronously; halving the calls
                # halves the convoy wait when a pair is still busy.
                halves = []  # (cl, half, off, n, qnb_half)
                for cl in p8_calls:
                    qnb = cl["qnb"]
                    for h in (0, 1):
                        qh = qnb[2 * h : 2 * h + 2]
                        nh = sum(qh) * P
                        if nh == 0:
                            continue
                        offh = cl["off"] + sum(qnb[: 2 * h]) * P
                        halves.append((cl, h, offh, nh, qh))
                halves.sort(key=lambda t: (t[1], t[0]["ch"] == 0, t[0]["ch"]))
                seg_by_ch = {}
                for cl, h, offh, nh, qh in halves:
                    # queues 1-3 only: the Pool NX's completion handshake for
                    # EVERY instruction needs cpus 0-1 to pop it, so any work
                    # on queue 0 (= Q7 pair 0) would serialize all dispatches
                    nc._gq = 1 + (getattr(nc, "_gq", 0)) % 3
                    nb = nh // P
                    it = idxp.tile([P, nh // 16], i16, tag="idx")
                    nc.sync.dma_start(
                        it[:], idx_p[:, offh // 16 : (offh + nh) // 16]
                    )
                    gt = gp.tile([P, nb, in_ch], bf16, tag="g")
                    ch0 = cl["ch"] * cfg.chunk
                    ch1 = min(ch0 + cfg.chunk, cfg.n)
                    nc.gpsimd.dma_gather(
                        gt[:], xs_full[ch0:ch1, :], it[:], nh, nh, in_ch,
                        single_packet=False, queue_num=nc._gq,
                    )
                    rel_sb = idxp.tile([P, nb], f32, tag="rel")
                    nc.sync.dma_start(
                        rel_sb[:], rel_p[:, offh // P : offh // P + nb]
                    )
                    seg_by_ch.setdefault(cl["ch"], {})[h] = (gt, rel_sb, qh)

                nquad = min(spc, cfg.nsw - spc * p8)
                disp_order = [c for c in (1, 2, 3, 0) if c in seg_by_ch]
                for qi in range(nquad):
                    sw = spc * p8 + qi
                    ws = list(range(sw * cfg.sww, min((sw + 1) * cfg.sww, nw)))
                    h, hq = divmod(qi, 2)
                    # consume chunks in gather-landing order (queue 0 lands
                    # last: its dispatch is the blocking one)
                    qseg = [seg_by_ch[c][h] for c in disp_order if h in seg_by_ch[c]]
                    nmm = sum(
                        qh[hq] if hq < len(qh) else 0 for *_, qh in qseg
                    )
                    psw = pswp.tile([P, wb], f32, space="PSUM", tag="psw")
                    # self-loop injection; only the FIRST matmul of the tile
                    # sets start=True (zero_accum zeroes the whole tile)
                    for wi, w in enumerate(ws):
                        nc.tensor.matmul(
                            psw[:, wi * P : (wi + 1) * P],
                            lhsT=xs_sb[:, w, :],
                            rhs=ident_bf[:],
                            start=(wi == 0),
                            stop=(nmm == 0 and wi == len(ws) - 1),
                            skip_group_check=True,
                        )
                    wsw = len(ws) * P
                    k = 0
                    for gt, rel_sb, qh in qseg:
                        qoff = sum(qh[:hq])
                        for bi in range(qoff, qoff + (qh[hq] if hq < len(qh) else 0)):
                            st = sp.tile([P, wsw], bf16, tag="s")
                            nc.vector.tensor_scalar(
                                st[:],
                                iota_f[:, :wsw],
                                rel_sb[:, bi : bi + 1],
                                None,
                                OP.is_equal,
                            )
                            k += 1
                            nc.tensor.matmul(
                                psw[:, :wsw],
                                lhsT=gt[:, bi, :],
                                rhs=st[:],
                                start=False,
                                stop=(k == nmm),
                                skip_group_check=True,
                            )
                    emit_epilogue(psw, ws)

    nc.compile()
    return nc


def _prep_inputs(x, edge_index, W, b, alpha, cfg):
    dinv, core_arrays, structure = route(edge_index, cfg)
    x = np.asarray(x, dtype=np.float32)
    W = np.asarray(W, dtype=np.float32)
    b = np.asarray(b, dtype=np.float32).reshape(1, cfg.hid)
    alpha = np.asarray(alpha, dtype=np.float32).reshape(1, 1)

    pad_n = cfg.nw * P - cfg.nsh
    in_maps = []
    for c in range(cfg.cores):
        idx_w, rel_w = core_arrays[c]
        dsh = dinv[c * cfg.nsh : (c + 1) * cfg.nsh]
        dsh = np.concatenate([dsh, np.ones(pad_n, np.float32)])
        in_maps.append(
            {
                "x_sh": x[c * cfg.nsh : (c + 1) * cfg.nsh],
                "dinv_t": dsh.reshape(cfg.nw, P).T.copy(),
                "sqdeg_r": (1.0 / dsh).reshape(1, cfg.nw * P).astype(np.float16),
                "w_p": W,
                "b_p": b,
                "alpha_p": alpha,
                "idx_p": idx_w,
                "rel_p": rel_w,
            }
        )
    return in_maps, structure


def kernel(x, edge_index, W, b, alpha):
    from concourse.bass_utils import run_bass_kernel_spmd

    cfg = CFG
    in_maps, structure = _prep_inputs(x, edge_index, W, b, alpha, cfg)
    nc = build(cfg, structure)
    res = run_bass_kernel_spmd(nc, in_maps, list(range(cfg.cores)))
    out = np.concatenate(
        [np.asarray(res.results[c]["out"]) for c in range(cfg.cores)], axis=0
    )
    return out.astype(np.float32)
